# revision 6
# baseline (speedup 1.0000x reference)
"""Windowed multi-head attention TRN2 kernel (Bass/Tile), SPMD over 8 cores.

Problem (per reference): x:(8,512,64,64) viewed as (B, 4096 tok, 512 c);
Q/K/V = tok @ W^T + b; per window (64 tok) & head (8 x 64d):
softmax(QK^T/8 + Bbias) @ V; output back in (B,512,64,64).

Sharding: data-parallel, one batch element per core (8 cores).

Per-core dataflow (all matmuls fp16 operands, fp32 PSUM accum):
 - host passes x^T (c, tok) fp16 so projection rhs tiles DMA contiguously
 - Q^T,K^T computed in [c_out, tok] layout (heads pairs on partition halves)
 - V computed in natural [tok, c] layout, with a per-head ones-column
   appended (65-wide head blocks) so PV matmuls also produce softmax
   denominators; V duplicated to both partition halves (Vdup) so any
   (window-parity, head-parity) combination is contraction-co-located
 - scores^T = K^T_wh^T-matmul: [k,q] tiles packed 8 units/PSUM bank
 - softmax without max-subtraction (scores are O(1)): exp on ACT,
   bias folded as elementwise exp(Bbias^T) multiply on DVE
 - PV: out[q, d|sum]; normalize by reciprocal(sums) during PSUM->SBUF
   evacuation on DVE; store natural [tok, c] tiles straight to HBM
"""

import sys
import numpy as np

for _p in ("/opt/trn_rl_repo",):
    if _p not in sys.path:
        sys.path.insert(0, _p)

from contextlib import ExitStack

import concourse.bass as bass
import concourse.tile as tile
from concourse import mybir

F16 = mybir.dt.float16
F32 = mybir.dt.float32

B, C, HH, WW = 8, 512, 64, 64
NH, HD = 8, 64
WIN = 64            # tokens per window
TOK = C * 0 + 4096  # tokens per batch/core
NT = 8              # 512-token tiles per core
NCHUNK = 4          # 128-channel chunks

TRACE = False
LAST = {}


def _emit(tc, out, xT, wq, wk, wv, ebt, bqk, iters=1):
    """Emit the per-core program. bqk: [128, 8] fp32 (bq/8 | bk chunks) or None."""
    nc = tc.nc
    Exp = mybir.ActivationFunctionType.Exp
    Ident = mybir.ActivationFunctionType.Identity

    with ExitStack() as ctx:
        ep = ctx.enter_context

        wpool = ep(tc.tile_pool(name="w", bufs=1))
        xpool = ep(tc.tile_pool(name="x", bufs=2))
        qkpool = ep(tc.tile_pool(name="qk", bufs=2))
        vpool = ep(tc.tile_pool(name="v", bufs=2))
        epool = ep(tc.tile_pool(name="e", bufs=2))
        rcpool = ep(tc.tile_pool(name="rc", bufs=8))
        onpool = ep(tc.tile_pool(name="on", bufs=2))
        projps = ep(tc.tile_pool(name="projps", bufs=2, space="PSUM"))
        sps = ep(tc.tile_pool(name="sps", bufs=2, space="PSUM"))
        ops = ep(tc.tile_pool(name="ops", bufs=4, space="PSUM"))

        # resident weights: [c_in chunk 128, c_out 512] fp16 per proj
        wsb = {}
        for nm, wdram in (("q", wq), ("k", wk), ("v", wv)):
            for ci in range(NCHUNK):
                t = wpool.tile([128, 512], F16, tag=f"w{nm}{ci}")
                nc.sync.dma_start(t[:], wdram[ci * 128:(ci + 1) * 128, :])
                wsb[nm, ci] = t
        ebt_sb = wpool.tile([128, 64], F16, tag="ebt")
        nc.sync.dma_start(ebt_sb[:], ebt[:, :])
        bqk_sb = None
        if bqk is not None:
            bqk_sb = wpool.tile([128, 8], F32, tag="bqk")
            nc.sync.dma_start(bqk_sb[:], bqk[:, :])

        if iters > 1:
            # on-device repetition for timing: amortizes host dispatch
            ep(tc.For_i(0, iters))

        for T in range(NT):
            # ---- load x^T chunks [c_in 128, tok 512]
            xt = []
            for ci in range(NCHUNK):
                t = xpool.tile([128, 512], F16, tag=f"xt{ci}")
                nc.sync.dma_start(
                    t[:], xT[ci * 128:(ci + 1) * 128, T * 512:(T + 1) * 512])
                xt.append(t)

            # ---- Q^T / K^T projections -> [c_out 128, tok 512] fp16
            qkt = {}
            for pi, nm in enumerate(("q", "k")):
                for co in range(NCHUNK):
                    ps = projps.tile([128, 512], F32, tag="proj")
                    for ci in range(NCHUNK):
                        nc.tensor.matmul(
                            ps[:],
                            wsb[nm, ci][:, co * 128:(co + 1) * 128],
                            xt[ci][:],
                            start=(ci == 0), stop=(ci == NCHUNK - 1))
                    t = qkpool.tile([128, 512], F16, tag=f"{nm}t{co}")
                    if bqk_sb is not None:
                        nc.scalar.activation(
                            t[:], ps[:], Ident,
                            bias=bqk_sb[:, pi * 4 + co:pi * 4 + co + 1])
                    else:
                        nc.scalar.copy(t[:], ps[:])
                    qkt[nm, co] = t

            # ---- V natural projection per 128-tok subtile -> Vnat
            vnat = []
            for tt in range(NCHUNK):
                ps = projps.tile([128, 512], F32, tag="proj")
                for ci in range(NCHUNK):
                    nc.tensor.matmul(
                        ps[:],
                        xt[ci][:, tt * 128:(tt + 1) * 128],
                        wsb["v", ci][:],
                        start=(ci == 0), stop=(ci == NCHUNK - 1))
                vn = vpool.tile([128, 520], F16, tag=f"vn{tt}")
                vn_v = vn[:].rearrange("p (h x) -> p h x", x=65)
                nc.scalar.activation(
                    vn_v[:, :, 64], ebt_sb[:, 0:8], Ident, bias=1.0, scale=0.0)
                nc.scalar.copy(
                    vn_v[:, :, 0:64],
                    ps[:].rearrange("p (h x) -> p h x", x=64))
                # row-swapped copy: window-odd V at rows 0:64 / even at 64:128
                vd = vpool.tile([128, 520], F16, tag=f"vd{tt}")
                nc.sync.dma_start(vd[0:64, :], vn[64:128, :])
                nc.sync.dma_start(vd[64:128, :], vn[0:64, :])
                vnat.append((vn, vd))

            # ---- attention: subtile tt covers windows 2tt, 2tt+1 of this T.
            # HAZARD RULE: concurrent matmuls with disjoint row-groups but a
            # shared column-group collide in the PE array (device crash), so
            # every sub-128 matmul here is placed DIAGONALLY: out partition
            # base == operand partition base (e*64). S/P and O tiles are
            # therefore head-parity packed (e), with Vdup supplying V rows at
            # the opposite parity and the normalize-evacuation shifting
            # partitions back to window-parity placement.
            for tt in range(NCHUNK):
                s = sps.tile([128, 512], F32, tag="s")
                for j in range(4):
                    for e in range(2):
                        r = slice(e * 64, e * 64 + 64)
                        c = j * 2 + e
                        for p in range(2):
                            w = 2 * tt + p
                            wc = slice(w * 64, w * 64 + 64)
                            nc.tensor.matmul(
                                s[r, (j * 2 + p) * 64:(j * 2 + p + 1) * 64],
                                qkt["k", j][r, wc],
                                qkt["q", j][r, wc],
                                start=True, stop=True)
                et = epool.tile([128, 512], F16, tag="et")
                nc.scalar.activation(et[:], s[:], Exp)
                pt = epool.tile([128, 512], F16, tag="pt")
                nc.vector.tensor_mul(
                    pt[:].rearrange("p (u x) -> p u x", x=64),
                    et[:].rearrange("p (u x) -> p u x", x=64),
                    ebt_sb[:].unsqueeze(1).broadcast_to((128, 8, 64)))

                on = onpool.tile([128, 512], F32, tag=f"on{tt % 2}")
                for j in range(4):
                    for p in range(2):
                        g = p * 64
                        o = ops.tile([128, 130], F32, tag="o")
                        o_v = o[:].rearrange("p (e x) -> p e x", x=65)
                        rc = rcpool.tile([128, 2], F32, tag="rc")
                        for e in range(2):
                            h = 2 * j + e
                            re = slice(e * 64, e * 64 + 64)
                            vsrc = vnat[tt][0 if p == e else 1]
                            nc.tensor.matmul(
                                o[re, e * 65:(e + 1) * 65],
                                pt[re, (j * 2 + p) * 64:(j * 2 + p + 1) * 64],
                                vsrc[re, h * 65:(h + 1) * 65],
                                start=True, stop=True)
                        for e in range(2):
                            re = slice(e * 64, e * 64 + 64)
                            nc.vector.reciprocal(
                                rc[re, e:e + 1], o_v[re, e, 64:65])
                            nc.vector.tensor_mul(
                                on[g:g + 64,
                                   j * 128 + e * 64: j * 128 + (e + 1) * 64],
                                o_v[re, e, 0:64],
                                rc[re, e:e + 1].broadcast_to((64, 64)))
                nc.sync.dma_start(
                    out[T * 512 + tt * 128: T * 512 + (tt + 1) * 128, :], on[:])


def _legalize_sync(nc, max_waits=1):
    """Hoist excess semaphore waits into standalone same-engine
    EventSemaphore instructions. Engine instruction streams execute in
    order, so a wait carried by an immediately-preceding EventSemaphore is
    equivalent to a wait on the instruction itself — and the walrus build
    in this environment rejects instructions with more than one wait."""
    import bass_rust
    n_new = 0
    fn = nc.m.functions[0]
    for blk in fn.blocks:
        out = []
        changed = False
        for ins in blk.instructions:
            si = ins.sync_info
            waits = list(si.on_wait) if si and si.on_wait else []
            if len(waits) > max_waits:
                keep = waits[-max_waits:]
                for w in waits[:-max_waits]:
                    es = mybir.InstEventSemaphore(
                        name=f"esw-{n_new}-{ins.name}", ins=[], outs=[])
                    es.engine = ins.engine
                    es.sync_info = bass_rust.SyncInfo(on_wait=[w], on_update=[])
                    out.append(es)
                    n_new += 1
                ins.sync_info = bass_rust.SyncInfo(
                    on_wait=keep,
                    on_update=list(si.on_update) if si.on_update else [])
                changed = True
            out.append(ins)
        if changed:
            blk.instructions = out
    return n_new


def _build_model(with_bias, iters=1):
    nc = bass.Bass("TRN2", target_bir_lowering=False, debug=False,
                   enable_partition_id=False)
    xT = nc.dram_tensor("xT", [512, 4096], F16, kind="ExternalInput").ap()
    wq = nc.dram_tensor("wq", [512, 512], F16, kind="ExternalInput").ap()
    wk = nc.dram_tensor("wk", [512, 512], F16, kind="ExternalInput").ap()
    wv = nc.dram_tensor("wv", [512, 512], F16, kind="ExternalInput").ap()
    ebt = nc.dram_tensor("ebt", [128, 64], F16, kind="ExternalInput").ap()
    bqk = (nc.dram_tensor("bqk", [128, 8], F32, kind="ExternalInput").ap()
           if with_bias else None)
    out = nc.dram_tensor("out", [4096, 512], F32, kind="ExternalOutput").ap()
    with tile.TileContext(nc) as tc:
        _emit(tc, out, xT, wq, wk, wv, ebt, bqk, iters=iters)
    return nc


_MODEL_CACHE = {}


def get_model(with_bias=False, legalize=True, iters=1):
    key = (with_bias, legalize, iters)
    if key not in _MODEL_CACHE:
        nc = _build_model(with_bias, iters=iters)
        if legalize:
            _legalize_sync(nc)
        _MODEL_CACHE[key] = nc
    return _MODEL_CACHE[key]


def make_in_maps(x, Wq, bq, Wk, bk, Wv, bv, Bbias):
    """Host-side sharding + layout prep. Returns (in_maps, with_bias)."""
    x = np.asarray(x, np.float32)
    with_bias = bool(np.any(bq) or np.any(bk))
    if np.any(bv):
        raise NotImplementedError("nonzero bv not supported")
    wq16 = np.ascontiguousarray(np.asarray(Wq, np.float32).T / 8.0).astype(np.float16)
    wk16 = np.ascontiguousarray(np.asarray(Wk, np.float32).T).astype(np.float16)
    wv16 = np.ascontiguousarray(np.asarray(Wv, np.float32).T).astype(np.float16)
    eb = np.exp(np.asarray(Bbias, np.float32).T)
    ebt = np.concatenate([eb, eb], 0).astype(np.float16)  # [128 (k x2), 64 q]
    common = {"wq": wq16, "wk": wk16, "wv": wv16, "ebt": ebt}
    if with_bias:
        bqk = np.concatenate(
            [np.asarray(bq, np.float32).reshape(4, 128).T / 8.0,
             np.asarray(bk, np.float32).reshape(4, 128).T], 1)  # [128, 8]
        common["bqk"] = np.ascontiguousarray(bqk)
    in_maps = []
    for b in range(B):
        xT16 = np.ascontiguousarray(
            x[b].reshape(TOK, C).T).astype(np.float16)
        in_maps.append({"xT": xT16, **common})
    return in_maps, with_bias


def kernel(**inputs):
    from concourse.bass_utils import run_bass_kernel_spmd
    in_maps, with_bias = make_in_maps(**inputs)
    nc = get_model(with_bias)
    res = run_bass_kernel_spmd(
        nc, in_maps, core_ids=list(range(B)), trace=TRACE)
    LAST["results"] = res
    out = np.stack([r["out"] for r in res.results], 0)
    return out.reshape(B, C, HH, WW)


def _harvest_io(nc):
    import jax
    in_names, out_names, out_avals = [], [], []
    for alloc in nc.m.functions[0].allocations:
        if not isinstance(alloc, mybir.MemoryLocationSet):
            continue
        name = alloc.memorylocations[0].name
        if alloc.kind == "ExternalInput":
            in_names.append(name)
        elif alloc.kind == "ExternalOutput":
            out_names.append(name)
            out_avals.append(jax.core.ShapedArray(
                tuple(alloc.tensor_shape), mybir.dt.np(alloc.dtype)))
    return in_names, out_names, out_avals


def _make_timed_callable(nc, in_maps):
    """Build a jitted shard_map callable around the single bass_exec of
    `nc` (mirrors run_bass_via_pjrt, but with NO donation so the same
    device-resident args can be reused across timed calls; outputs are
    garbage — timing only). Returns a zero-arg closure that runs one
    dispatch and blocks."""
    import jax
    from jax.sharding import Mesh, PartitionSpec
    from jax.experimental.shard_map import shard_map
    from concourse import bass2jax

    bass2jax.install_neuronx_cc_hook()
    in_names, out_names, out_avals = _harvest_io(nc)
    n_params = len(in_names)
    all_names = tuple(in_names + out_names)
    n_cores = len(in_maps)

    def _body(*args):
        return tuple(bass2jax._bass_exec_p.bind(
            *args,
            out_avals=tuple(out_avals),
            in_names=all_names,
            out_names=tuple(out_names),
            lowering_input_output_aliases=(),
            sim_require_finite=True,
            sim_require_nnan=True,
            nc=nc))

    devices = jax.devices()[:n_cores]
    mesh = Mesh(np.asarray(devices), ("core",))
    n_all = n_params + len(out_names)
    sharded = jax.jit(shard_map(
        _body, mesh=mesh,
        in_specs=(PartitionSpec("core"),) * n_all,
        out_specs=(PartitionSpec("core"),) * len(out_names),
        check_rep=False), keep_unused=True)
    concat_in = [
        np.concatenate([np.asarray(m[name]) for m in in_maps], 0)
        for name in in_names]
    concat_zeros = [
        np.zeros((n_cores * a.shape[0], *a.shape[1:]), a.dtype)
        for a in out_avals]
    args = [jax.device_put(a) for a in concat_in + concat_zeros]
    jax.block_until_ready(sharded(*args))  # warm-up / compile

    def run():
        jax.block_until_ready(sharded(*args))
    return run


def time_kernel(inputs, iters=64, samples=12):
    """Returns ns per iteration. Builds two model variants — the body run
    once vs `1+iters` times inside an on-device For_i loop — and
    differences min wall-clock over `samples` dispatches of each, which
    cancels axon dispatch + host overhead."""
    import time
    in_maps, with_bias = make_in_maps(**inputs)
    run1 = _make_timed_callable(get_model(with_bias, iters=1), in_maps)
    runN = _make_timed_callable(
        get_model(with_bias, iters=1 + iters), in_maps)
    t1s, tNs = [], []
    for _ in range(samples):
        t0 = time.time(); run1(); t1s.append(time.time() - t0)
        t0 = time.time(); runN(); tNs.append(time.time() - t0)
    t1, tN = min(t1s), min(tNs)
    return (tN - t1) / iters * 1e9, (t1s, tNs)



# revision 17
# speedup vs baseline: 3.7619x; 3.7619x over previous
"""Windowed multi-head attention TRN2 kernel (Bass/Tile), SPMD over 8 cores.

Problem (per reference): x:(8,512,64,64) viewed as (B, 4096 tok, 512 c);
Q/K/V = tok @ W^T + b; per window (64 tok) & head (8 x 64d):
softmax(QK^T/8 + Bbias) @ V; output back in (B,512,64,64).

Sharding: data-parallel, one batch element per core (8 cores).

Per-core dataflow (all matmuls fp16 operands, fp32 PSUM accum):
 - host passes x^T (c, tok) fp16 so projection rhs tiles DMA contiguously
 - Q^T,K^T computed in [c_out, tok] layout (heads pairs on partition halves)
 - V computed in natural [tok, c] layout, with a per-head ones-column
   appended (65-wide head blocks) so PV matmuls also produce softmax
   denominators
 - scores^T = K^T_wh^T-matmul: [k,q] tiles packed 8 units/PSUM bank,
   head-parity (e) on partition halves
 - softmax without max-subtraction (scores are O(1)): exp on ACT; the
   exp(Bbias^T) elementwise multiply on DVE simultaneously moves probs
   to window-parity (p) partition halves, so PV runs against natural V
   (no duplicate) and outputs land in natural token rows
 - PV: [64q, 65] units, 4 per PSUM bank; normalize via one batched
   reciprocal + one batched multiply per bank during PSUM->SBUF evac
"""

import sys
import numpy as np

for _p in ("/opt/trn_rl_repo",):
    if _p not in sys.path:
        sys.path.insert(0, _p)

from contextlib import ExitStack

import concourse.bass as bass
import concourse.tile as tile
from concourse import mybir

F16 = mybir.dt.float16
F32 = mybir.dt.float32

B, C, HH, WW = 8, 512, 64, 64
NH, HD = 8, 64
WIN = 64            # tokens per window
TOK = C * 0 + 4096  # tokens per batch/core
NT = 8              # 512-token tiles per core
NCHUNK = 4          # 128-channel chunks

TRACE = False
LAST = {}


def _emit(tc, out, xT, wq, wk, wv, ebt, bqk, iters=1, parts="pas"):
    """Emit the per-core program. bqk: [128, 8] fp32 (bq/8 | bk chunks) or None.
    parts: subset of 'p' (projections), 'a' (attention), 's' (store) for
    timing ablations."""
    nc = tc.nc
    Exp = mybir.ActivationFunctionType.Exp
    Ident = mybir.ActivationFunctionType.Identity

    with ExitStack() as ctx:
        ep = ctx.enter_context

        wpool = ep(tc.tile_pool(name="w", bufs=1))
        xpool = ep(tc.tile_pool(name="x", bufs=2))
        qkpool = ep(tc.tile_pool(name="qk", bufs=2))
        vpool = ep(tc.tile_pool(name="v", bufs=2))
        epool = ep(tc.tile_pool(name="e", bufs=2))
        rcpool = ep(tc.tile_pool(name="rc", bufs=4))
        onpool = ep(tc.tile_pool(name="on", bufs=2))
        projps = ep(tc.tile_pool(name="projps", bufs=2, space="PSUM"))
        sps = ep(tc.tile_pool(name="sps", bufs=2, space="PSUM"))
        ops = ep(tc.tile_pool(name="ops", bufs=2, space="PSUM"))

        # resident weights: [c_in chunk 128, c_out 512] fp16 per proj
        wsb = {}
        for nm, wdram in (("q", wq), ("k", wk), ("v", wv)):
            for ci in range(NCHUNK):
                t = wpool.tile([128, 512], F16, tag=f"w{nm}{ci}")
                nc.sync.dma_start(t[:], wdram[ci * 128:(ci + 1) * 128, :])
                wsb[nm, ci] = t
        ebt_sb = wpool.tile([128, 64], F16, tag="ebt")
        nc.sync.dma_start(ebt_sb[:], ebt[:, :])
        bqk_sb = None
        if bqk is not None:
            bqk_sb = wpool.tile([128, 8], F32, tag="bqk")
            nc.sync.dma_start(bqk_sb[:], bqk[:, :])

        if iters > 1:
            # on-device repetition for timing: amortizes host dispatch
            ep(tc.For_i(0, iters))

        for T in range(NT):
            # ---- load x^T chunks [c_in 128, tok 512]
            xt = []
            for ci in range(NCHUNK):
                t = xpool.tile([128, 512], F16, tag=f"xt{ci}")
                nc.sync.dma_start(
                    t[:], xT[ci * 128:(ci + 1) * 128, T * 512:(T + 1) * 512])
                xt.append(t)

            # ---- Q^T / K^T projections -> [c_out 128, tok 512] fp16
            qkt = {}
            for pi, nm in enumerate(("q", "k")):
                for co in range(NCHUNK):
                    t = qkpool.tile([128, 512], F16, tag=f"{nm}t{co}")
                    qkt[nm, co] = t
                    if "p" not in parts:
                        continue
                    ps = projps.tile([128, 512], F32, tag="proj")
                    for ci in range(NCHUNK):
                        nc.tensor.matmul(
                            ps[:],
                            wsb[nm, ci][:, co * 128:(co + 1) * 128],
                            xt[ci][:],
                            start=(ci == 0), stop=(ci == NCHUNK - 1))
                    if bqk_sb is not None:
                        nc.scalar.activation(
                            t[:], ps[:], Ident,
                            bias=bqk_sb[:, pi * 4 + co:pi * 4 + co + 1])
                    else:
                        nc.scalar.copy(t[:], ps[:])

            # ---- V natural projection per 128-tok subtile -> vn
            # vn[p*64+k, h*65+d] = V[token tt*128+p*64+k, channel h*64+d];
            # col h*65+64 holds 1.0 (written once per physical buffer: the
            # PV matmul then yields softmax denominators for free).
            vnat = []
            for tt in range(NCHUNK):
                vn = vpool.tile([128, 520], F16, tag=f"vn{tt}")
                vnat.append(vn)
                if "p" not in parts:
                    continue
                vn_v = vn[:].rearrange("p (h x) -> p h x", x=65)
                if T < 2:
                    nc.scalar.activation(
                        vn_v[:, :, 64], ebt_sb[:, 0:8], Ident,
                        bias=1.0, scale=0.0)
                ps = projps.tile([128, 512], F32, tag="proj")
                for ci in range(NCHUNK):
                    nc.tensor.matmul(
                        ps[:],
                        xt[ci][:, tt * 128:(tt + 1) * 128],
                        wsb["v", ci][:],
                        start=(ci == 0), stop=(ci == NCHUNK - 1))
                nc.vector.tensor_copy(
                    vn_v[:, :, 0:64],
                    ps[:].rearrange("p (h x) -> p h x", x=64))

            # ---- attention: subtile tt covers windows 2tt, 2tt+1 of this T.
            # HAZARD RULE: concurrent matmuls with disjoint row-groups but a
            # shared column-group collide in the PE array (device crash), so
            # every sub-128 matmul is placed DIAGONALLY: out partition base
            # == operand partition base. Scores are head-parity packed (e on
            # halves, forced by qkt layout); the exp(Bbias)-multiply on DVE
            # also moves probs to window-parity rows (p on halves), so PV
            # runs against natural-layout V (no duplicate) and the output
            # lands directly in natural token rows.
            for tt in range(NCHUNK):
                if "a" not in parts:
                    break
                s = sps.tile([128, 512], F32, tag="s")
                for j in range(4):
                    for e in range(2):
                        r = slice(e * 64, e * 64 + 64)
                        for p in range(2):
                            w = 2 * tt + p
                            wc = slice(w * 64, w * 64 + 64)
                            nc.tensor.matmul(
                                s[r, (j * 2 + p) * 64:(j * 2 + p + 1) * 64],
                                qkt["k", j][r, wc],
                                qkt["q", j][r, wc],
                                start=True, stop=True)
                et = epool.tile([128, 512], F16, tag="et")
                nc.scalar.activation(et[:], s[:], Exp)
                # pt[p*64+k, (2j+e)*64+q] = et[e*64+k, (2j+p)*64+q]*ebt[k,q]
                pt = epool.tile([128, 512], F16, tag="pt")
                et_v = et[:].rearrange("r (j u q) -> r j u q", u=2, q=64)
                pt_v = pt[:].rearrange("r (j u q) -> r j u q", u=2, q=64)
                for p in range(2):
                    rp = slice(p * 64, p * 64 + 64)
                    for e in range(2):
                        re = slice(e * 64, e * 64 + 64)
                        nc.vector.tensor_mul(
                            pt_v[rp, :, e, :],
                            et_v[re, :, p, :],
                            ebt_sb[re, 0:64].unsqueeze(1)
                            .broadcast_to((64, 4, 64)))

                # PV: two PSUM banks per tt, each holding (j2, e) units of
                # [64q, 65] for both window parities on partition halves.
                on = onpool.tile([128, 512], F32, tag=f"on{tt % 2}")
                for b in range(2):
                    o = ops.tile([128, 260], F32, tag=f"ob{b}")
                    o_v = o[:].rearrange("r (u x) -> r u x", x=65)
                    for j2 in range(2):
                        j = 2 * b + j2
                        for e in range(2):
                            h = 2 * j + e
                            u = j2 * 2 + e
                            for p in range(2):
                                rp = slice(p * 64, p * 64 + 64)
                                nc.tensor.matmul(
                                    o[rp, u * 65:(u + 1) * 65],
                                    pt[rp, h * 64:(h + 1) * 64],
                                    vnat[tt][rp, h * 65:(h + 1) * 65],
                                    start=True, stop=True)
                    rc = rcpool.tile([128, 4], F32, tag=f"rc{b}")
                    nc.vector.reciprocal(rc[:, 0:4], o_v[:, :, 64])
                    nc.vector.tensor_mul(
                        on[:].rearrange("r (b2 u q) -> r b2 u q", b2=2, q=64)
                        [:, b, :, :],
                        o_v[:, :, 0:64],
                        rc[:, 0:4].unsqueeze(2).broadcast_to((128, 4, 64)))
                if "s" in parts:
                    nc.sync.dma_start(
                        out[T * 512 + tt * 128: T * 512 + (tt + 1) * 128, :],
                        on[:])


def _legalize_sync(nc, max_waits=1):
    """Hoist excess semaphore waits into standalone same-engine
    EventSemaphore instructions. Engine instruction streams execute in
    order, so a wait carried by an immediately-preceding EventSemaphore is
    equivalent to a wait on the instruction itself — and the walrus build
    in this environment rejects instructions with more than one wait."""
    import bass_rust
    n_new = 0
    fn = nc.m.functions[0]
    for blk in fn.blocks:
        out = []
        changed = False
        for ins in blk.instructions:
            si = ins.sync_info
            waits = list(si.on_wait) if si and si.on_wait else []
            if len(waits) > max_waits:
                keep = waits[-max_waits:]
                for w in waits[:-max_waits]:
                    es = mybir.InstEventSemaphore(
                        name=f"esw-{n_new}-{ins.name}", ins=[], outs=[])
                    es.engine = ins.engine
                    es.sync_info = bass_rust.SyncInfo(on_wait=[w], on_update=[])
                    out.append(es)
                    n_new += 1
                ins.sync_info = bass_rust.SyncInfo(
                    on_wait=keep,
                    on_update=list(si.on_update) if si.on_update else [])
                changed = True
            out.append(ins)
        if changed:
            blk.instructions = out
    return n_new


def _build_model(with_bias, iters=1, parts="pas"):
    nc = bass.Bass("TRN2", target_bir_lowering=False, debug=False,
                   enable_partition_id=False)
    xT = nc.dram_tensor("xT", [512, 4096], F16, kind="ExternalInput").ap()
    wq = nc.dram_tensor("wq", [512, 512], F16, kind="ExternalInput").ap()
    wk = nc.dram_tensor("wk", [512, 512], F16, kind="ExternalInput").ap()
    wv = nc.dram_tensor("wv", [512, 512], F16, kind="ExternalInput").ap()
    ebt = nc.dram_tensor("ebt", [128, 64], F16, kind="ExternalInput").ap()
    bqk = (nc.dram_tensor("bqk", [128, 8], F32, kind="ExternalInput").ap()
           if with_bias else None)
    out = nc.dram_tensor("out", [4096, 512], F32, kind="ExternalOutput").ap()
    with tile.TileContext(nc) as tc:
        _emit(tc, out, xT, wq, wk, wv, ebt, bqk, iters=iters, parts=parts)
    return nc


_MODEL_CACHE = {}


def get_model(with_bias=False, legalize=True, iters=1, parts="pas"):
    key = (with_bias, legalize, iters, parts)
    if key not in _MODEL_CACHE:
        nc = _build_model(with_bias, iters=iters, parts=parts)
        if legalize:
            _legalize_sync(nc)
        _MODEL_CACHE[key] = nc
    return _MODEL_CACHE[key]


def make_in_maps(x, Wq, bq, Wk, bk, Wv, bv, Bbias):
    """Host-side sharding + layout prep. Returns (in_maps, with_bias)."""
    x = np.asarray(x, np.float32)
    with_bias = bool(np.any(bq) or np.any(bk))
    if np.any(bv):
        raise NotImplementedError("nonzero bv not supported")
    wq16 = np.ascontiguousarray(np.asarray(Wq, np.float32).T / 8.0).astype(np.float16)
    wk16 = np.ascontiguousarray(np.asarray(Wk, np.float32).T).astype(np.float16)
    wv16 = np.ascontiguousarray(np.asarray(Wv, np.float32).T).astype(np.float16)
    eb = np.exp(np.asarray(Bbias, np.float32).T)
    ebt = np.concatenate([eb, eb], 0).astype(np.float16)  # [128 (k x2), 64 q]
    common = {"wq": wq16, "wk": wk16, "wv": wv16, "ebt": ebt}
    if with_bias:
        bqk = np.concatenate(
            [np.asarray(bq, np.float32).reshape(4, 128).T / 8.0,
             np.asarray(bk, np.float32).reshape(4, 128).T], 1)  # [128, 8]
        common["bqk"] = np.ascontiguousarray(bqk)
    in_maps = []
    for b in range(B):
        xT16 = np.ascontiguousarray(
            x[b].reshape(TOK, C).T).astype(np.float16)
        in_maps.append({"xT": xT16, **common})
    return in_maps, with_bias


def kernel(**inputs):
    from concourse.bass_utils import run_bass_kernel_spmd
    in_maps, with_bias = make_in_maps(**inputs)
    nc = get_model(with_bias)
    res = run_bass_kernel_spmd(
        nc, in_maps, core_ids=list(range(B)), trace=TRACE)
    LAST["results"] = res
    out = np.stack([r["out"] for r in res.results], 0)
    return out.reshape(B, C, HH, WW)


def _harvest_io(nc):
    import jax
    in_names, out_names, out_avals = [], [], []
    for alloc in nc.m.functions[0].allocations:
        if not isinstance(alloc, mybir.MemoryLocationSet):
            continue
        name = alloc.memorylocations[0].name
        if alloc.kind == "ExternalInput":
            in_names.append(name)
        elif alloc.kind == "ExternalOutput":
            out_names.append(name)
            out_avals.append(jax.core.ShapedArray(
                tuple(alloc.tensor_shape), mybir.dt.np(alloc.dtype)))
    return in_names, out_names, out_avals


def _make_timed_callable(nc, in_maps):
    """Build a jitted shard_map callable around the single bass_exec of
    `nc` (mirrors run_bass_via_pjrt, but with NO donation so the same
    device-resident args can be reused across timed calls; outputs are
    garbage — timing only). Returns a zero-arg closure that runs one
    dispatch and blocks."""
    import jax
    from jax.sharding import Mesh, PartitionSpec
    from jax.experimental.shard_map import shard_map
    from concourse import bass2jax

    bass2jax.install_neuronx_cc_hook()
    in_names, out_names, out_avals = _harvest_io(nc)
    n_params = len(in_names)
    all_names = tuple(in_names + out_names)
    n_cores = len(in_maps)

    def _body(*args):
        return tuple(bass2jax._bass_exec_p.bind(
            *args,
            out_avals=tuple(out_avals),
            in_names=all_names,
            out_names=tuple(out_names),
            lowering_input_output_aliases=(),
            sim_require_finite=True,
            sim_require_nnan=True,
            nc=nc))

    devices = jax.devices()[:n_cores]
    mesh = Mesh(np.asarray(devices), ("core",))
    n_all = n_params + len(out_names)
    sharded = jax.jit(shard_map(
        _body, mesh=mesh,
        in_specs=(PartitionSpec("core"),) * n_all,
        out_specs=(PartitionSpec("core"),) * len(out_names),
        check_rep=False), keep_unused=True)
    concat_in = [
        np.concatenate([np.asarray(m[name]) for m in in_maps], 0)
        for name in in_names]
    concat_zeros = [
        np.zeros((n_cores * a.shape[0], *a.shape[1:]), a.dtype)
        for a in out_avals]
    args = [jax.device_put(a) for a in concat_in + concat_zeros]
    jax.block_until_ready(sharded(*args))  # warm-up / compile

    def run():
        jax.block_until_ready(sharded(*args))
    return run


def time_kernel(inputs, iters=4096, samples=8, parts="pas"):
    """Returns ns per iteration. Builds two model variants — the body run
    once vs `1+iters` times inside an on-device For_i loop — and
    differences median wall-clock over `samples` dispatches of each. With
    ~1s on-device per N-iter dispatch, the ~±20ms axon dispatch jitter
    contributes <2% error."""
    import time
    in_maps, with_bias = make_in_maps(**inputs)
    run1 = _make_timed_callable(
        get_model(with_bias, iters=1, parts=parts), in_maps)
    runN = _make_timed_callable(
        get_model(with_bias, iters=1 + iters, parts=parts), in_maps)
    t1s, tNs = [], []
    for _ in range(samples):
        t0 = time.time(); run1(); t1s.append(time.time() - t0)
        t0 = time.time(); runN(); tNs.append(time.time() - t0)
    t1 = float(np.median(t1s)); tN = float(np.median(tNs))
    return (tN - t1) / iters * 1e9, (t1s, tNs)



# revision 23
# speedup vs baseline: 3.9315x; 1.0451x over previous
"""Windowed multi-head attention TRN2 kernel (Bass/Tile), SPMD over 8 cores.

Problem (per reference): x:(8,512,64,64) viewed as (B, 4096 tok, 512 c);
Q/K/V = tok @ W^T + b; per window (64 tok) & head (8 x 64d):
softmax(QK^T/8 + Bbias) @ V; output back in (B,512,64,64).

Sharding: data-parallel, one batch element per core (8 cores).

Per-core dataflow (all matmuls fp16 operands, fp32 PSUM accum):
 - host passes x^T (c, tok) fp16 so projection rhs tiles DMA contiguously
 - Q^T,K^T computed in [c_out, tok] layout (heads pairs on partition halves)
 - V computed in natural [tok, c] layout, with a per-head ones-column
   appended (65-wide head blocks) so PV matmuls also produce softmax
   denominators
 - scores^T = K^T_wh^T-matmul: [k,q] tiles packed 8 units/PSUM bank,
   head-parity (e) on partition halves
 - softmax without max-subtraction (scores are O(1)): exp on ACT; the
   exp(Bbias^T) elementwise multiply on DVE simultaneously moves probs
   to window-parity (p) partition halves, so PV runs against natural V
   (no duplicate) and outputs land in natural token rows
 - PV: [64q, 65] units, 4 per PSUM bank; normalize via one batched
   reciprocal + one batched multiply per bank during PSUM->SBUF evac
"""

import sys
import numpy as np

for _p in ("/opt/trn_rl_repo",):
    if _p not in sys.path:
        sys.path.insert(0, _p)

from contextlib import ExitStack

import concourse.bass as bass
import concourse.tile as tile
from concourse import mybir

F16 = mybir.dt.float16
F32 = mybir.dt.float32

B, C, HH, WW = 8, 512, 64, 64
NH, HD = 8, 64
WIN = 64            # tokens per window
TOK = C * 0 + 4096  # tokens per batch/core
NT = 8              # 512-token tiles per core
NCHUNK = 4          # 128-channel chunks

TRACE = False
LAST = {}
SCORES_BD = True  # scores via block-diagonal K (full 128-part contraction)


def _emit(tc, out, xT, wq, wk, wv, ebt, bqk, iters=1, parts="pas"):
    """Emit the per-core program. bqk: [128, 8] fp32 (bq/8 | bk chunks) or None.
    parts: subset of 'p' (projections), 'a' (attention), 's' (store) for
    timing ablations."""
    nc = tc.nc
    Exp = mybir.ActivationFunctionType.Exp
    Ident = mybir.ActivationFunctionType.Identity

    with ExitStack() as ctx:
        ep = ctx.enter_context

        wpool = ep(tc.tile_pool(name="w", bufs=1))
        xpool = ep(tc.tile_pool(name="x", bufs=2))
        qkpool = ep(tc.tile_pool(name="qk", bufs=2))
        vpool = ep(tc.tile_pool(name="v", bufs=2))
        epool = ep(tc.tile_pool(name="e", bufs=2))
        bdpool = ep(tc.tile_pool(name="bd", bufs=2))
        rcpool = ep(tc.tile_pool(name="rc", bufs=4))
        onpool = ep(tc.tile_pool(name="on", bufs=2))
        projps = ep(tc.tile_pool(name="projps", bufs=2, space="PSUM"))
        sps = ep(tc.tile_pool(name="sps", bufs=2, space="PSUM"))
        ops = ep(tc.tile_pool(name="ops", bufs=2, space="PSUM"))

        # resident weights: [c_in chunk 128, c_out 512] fp16 per proj
        wsb = {}
        for nm, wdram in (("q", wq), ("k", wk), ("v", wv)):
            for ci in range(NCHUNK):
                t = wpool.tile([128, 512], F16, tag=f"w{nm}{ci}")
                nc.sync.dma_start(t[:], wdram[ci * 128:(ci + 1) * 128, :])
                wsb[nm, ci] = t
        ebt_sb = wpool.tile([128, 64], F16, tag="ebt")
        nc.sync.dma_start(ebt_sb[:], ebt[:, :])
        bqk_sb = None
        if bqk is not None:
            bqk_sb = wpool.tile([128, 8], F32, tag="bqk")
            nc.sync.dma_start(bqk_sb[:], bqk[:, :])

        if iters > 1:
            # on-device repetition for timing: amortizes host dispatch
            ep(tc.For_i(0, iters))

        for T in range(NT):
            # ---- load x^T chunks [c_in 128, tok 512]
            xt = []
            for ci in range(NCHUNK):
                t = xpool.tile([128, 512], F16, tag=f"xt{ci}")
                nc.sync.dma_start(
                    t[:], xT[ci * 128:(ci + 1) * 128, T * 512:(T + 1) * 512])
                xt.append(t)

            # ---- Q^T / K^T projections -> [c_out 128, tok 512] fp16
            qkt = {}
            for pi, nm in enumerate(("q", "k")):
                for co in range(NCHUNK):
                    t = qkpool.tile([128, 512], F16, tag=f"{nm}t{co}")
                    qkt[nm, co] = t
                    if "p" not in parts:
                        continue
                    ps = projps.tile([128, 512], F32, tag="proj")
                    for ci in range(NCHUNK):
                        nc.tensor.matmul(
                            ps[:],
                            wsb[nm, ci][:, co * 128:(co + 1) * 128],
                            xt[ci][:],
                            start=(ci == 0), stop=(ci == NCHUNK - 1))
                    if bqk_sb is not None:
                        nc.scalar.activation(
                            t[:], ps[:], Ident,
                            bias=bqk_sb[:, pi * 4 + co:pi * 4 + co + 1])
                    else:
                        nc.scalar.copy(t[:], ps[:])

            # ---- V natural projection per 128-tok subtile -> vn
            # vn[p*64+k, h*65+d] = V[token tt*128+p*64+k, channel h*64+d];
            # col h*65+64 holds 1.0 (written once per physical buffer: the
            # PV matmul then yields softmax denominators for free).
            vnat = []
            for tt in range(NCHUNK):
                vn = vpool.tile([128, 520], F16, tag=f"vn{tt}")
                vnat.append(vn)
                if "p" not in parts:
                    continue
                vn_v = vn[:].rearrange("p (h x) -> p h x", x=65)
                if T < 2:
                    nc.scalar.activation(
                        vn_v[:, :, 64], ebt_sb[:, 0:8], Ident,
                        bias=1.0, scale=0.0)
                ps = projps.tile([128, 512], F32, tag="proj")
                for ci in range(NCHUNK):
                    nc.tensor.matmul(
                        ps[:],
                        xt[ci][:, tt * 128:(tt + 1) * 128],
                        wsb["v", ci][:],
                        start=(ci == 0), stop=(ci == NCHUNK - 1))
                nc.vector.tensor_copy(
                    vn_v[:, :, 0:64],
                    ps[:].rearrange("p (h x) -> p h x", x=64))

            # ---- attention: subtile tt covers windows 2tt, 2tt+1 of this T.
            # HAZARD RULE: concurrent matmuls with disjoint row-groups but a
            # shared column-group collide in the PE array (device crash), so
            # every sub-128 matmul is placed DIAGONALLY: out partition base
            # == operand partition base. Scores are head-parity packed (e on
            # halves, forced by qkt layout); the exp(Bbias)-multiply on DVE
            # also moves probs to window-parity rows (p on halves), so PV
            # runs against natural-layout V (no duplicate) and the output
            # lands directly in natural token rows.
            # block-diagonal K (when SCORES_BD): bdk_j[e*64+d,
            # tt*256+p*128+e*64+k] = K^T[d, k] of head 2j+e, window 2tt+p;
            # off-diagonal blocks stay zero (written once per physical
            # buffer). One scores matmul then covers BOTH heads of a window
            # with full 128-partition contraction: N=64 cycles vs 2x64.
            bdks = []
            if SCORES_BD and "a" in parts:
                for j in range(4):
                    bdk = bdpool.tile([128, 1024], F16, tag=f"bdk{j}")
                    bdks.append(bdk)
                    if T < 2:
                        nc.gpsimd.memset(bdk[:], 0)
                    bd_v = bdk[:].rearrange(
                        "r (tt p c) -> r tt p c", p=2, c=128)
                    for e in range(2):
                        re = slice(e * 64, e * 64 + 64)
                        nc.gpsimd.tensor_copy(
                            bd_v[re, :, :, e * 64:e * 64 + 64],
                            qkt["k", j][re, :]
                            .rearrange("r (tt p k) -> r tt p k", p=2, k=64))

            for tt in range(NCHUNK):
                if "a" not in parts:
                    break
                s = sps.tile([128, 512], F32, tag="s")
                if SCORES_BD:
                    for j in range(4):
                        for p in range(2):
                            w = 2 * tt + p
                            nc.tensor.matmul(
                                s[:, (j * 2 + p) * 64:(j * 2 + p + 1) * 64],
                                bdks[j][:, tt * 256 + p * 128:
                                        tt * 256 + (p + 1) * 128],
                                qkt["q", j][:, w * 64:(w + 1) * 64],
                                start=True, stop=True)
                else:
                    for j in range(4):
                        for e in range(2):
                            r = slice(e * 64, e * 64 + 64)
                            for p in range(2):
                                w = 2 * tt + p
                                wc = slice(w * 64, w * 64 + 64)
                                nc.tensor.matmul(
                                    s[r, (j * 2 + p) * 64:
                                      (j * 2 + p + 1) * 64],
                                    qkt["k", j][r, wc],
                                    qkt["q", j][r, wc],
                                    start=True, stop=True)
                et = epool.tile([128, 512], F16, tag="et")
                nc.scalar.activation(et[:], s[:], Exp)
                # block-diagonal probs: pt[p*64+k, h*128+p*64+q] =
                # et[e*64+k, (2j+p)*64+q]*ebt[k,q] (h=2j+e); off-diagonal
                # blocks stay zero, so one PV matmul covers both windows
                # with full 128-partition contraction against natural V.
                pt = epool.tile([128, 1024], F16, tag="pt")
                if T == 0 and tt < 2:
                    nc.gpsimd.memset(pt[:], 0)
                et_v = et[:].rearrange("r (j u q) -> r j u q", u=2, q=64)
                pt_v = pt[:].rearrange("r (j z) -> r j z", j=4)
                for p in range(2):
                    rp = slice(p * 64, p * 64 + 64)
                    for e in range(2):
                        re = slice(e * 64, e * 64 + 64)
                        c0 = e * 128 + p * 64
                        nc.vector.tensor_mul(
                            pt_v[rp, :, c0:c0 + 64],
                            et_v[re, :, p, :],
                            ebt_sb[re, 0:64].unsqueeze(1)
                            .broadcast_to((64, 4, 64)))

                # PV: 8 matmuls (one per head), full 128 partitions; two
                # PSUM banks of 4 [128q2w, 65] units each.
                on = onpool.tile([128, 512], F32, tag=f"on{tt % 2}")
                for b in range(2):
                    o = ops.tile([128, 260], F32, tag=f"ob{b}")
                    o_v = o[:].rearrange("r (u x) -> r u x", x=65)
                    for u in range(4):
                        h = 4 * b + u
                        nc.tensor.matmul(
                            o[:, u * 65:(u + 1) * 65],
                            pt[:, h * 128:(h + 1) * 128],
                            vnat[tt][:, h * 65:(h + 1) * 65],
                            start=True, stop=True)
                    rc = rcpool.tile([128, 4], F32, tag=f"rc{b}")
                    nc.vector.reciprocal(rc[:, 0:4], o_v[:, :, 64])
                    nc.vector.tensor_mul(
                        on[:].rearrange("r (b2 u q) -> r b2 u q", b2=2, q=64)
                        [:, b, :, :],
                        o_v[:, :, 0:64],
                        rc[:, 0:4].unsqueeze(2).broadcast_to((128, 4, 64)))
                if "s" in parts:
                    nc.sync.dma_start(
                        out[T * 512 + tt * 128: T * 512 + (tt + 1) * 128, :],
                        on[:])


def _legalize_sync(nc, max_waits=1):
    """Hoist excess semaphore waits into standalone same-engine
    EventSemaphore instructions. Engine instruction streams execute in
    order, so a wait carried by an immediately-preceding EventSemaphore is
    equivalent to a wait on the instruction itself — and the walrus build
    in this environment rejects instructions with more than one wait."""
    import bass_rust
    n_new = 0
    fn = nc.m.functions[0]
    for blk in fn.blocks:
        out = []
        changed = False
        for ins in blk.instructions:
            si = ins.sync_info
            waits = list(si.on_wait) if si and si.on_wait else []
            if len(waits) > max_waits:
                keep = waits[-max_waits:]
                for w in waits[:-max_waits]:
                    es = mybir.InstEventSemaphore(
                        name=f"esw-{n_new}-{ins.name}", ins=[], outs=[])
                    es.engine = ins.engine
                    es.sync_info = bass_rust.SyncInfo(on_wait=[w], on_update=[])
                    out.append(es)
                    n_new += 1
                ins.sync_info = bass_rust.SyncInfo(
                    on_wait=keep,
                    on_update=list(si.on_update) if si.on_update else [])
                changed = True
            out.append(ins)
        if changed:
            blk.instructions = out
    return n_new


def _build_model(with_bias, iters=1, parts="pas"):
    nc = bass.Bass("TRN2", target_bir_lowering=False, debug=False,
                   enable_partition_id=False)
    xT = nc.dram_tensor("xT", [512, 4096], F16, kind="ExternalInput").ap()
    wq = nc.dram_tensor("wq", [512, 512], F16, kind="ExternalInput").ap()
    wk = nc.dram_tensor("wk", [512, 512], F16, kind="ExternalInput").ap()
    wv = nc.dram_tensor("wv", [512, 512], F16, kind="ExternalInput").ap()
    ebt = nc.dram_tensor("ebt", [128, 64], F16, kind="ExternalInput").ap()
    bqk = (nc.dram_tensor("bqk", [128, 8], F32, kind="ExternalInput").ap()
           if with_bias else None)
    out = nc.dram_tensor("out", [4096, 512], F32, kind="ExternalOutput").ap()
    with tile.TileContext(nc) as tc:
        _emit(tc, out, xT, wq, wk, wv, ebt, bqk, iters=iters, parts=parts)
    return nc


_MODEL_CACHE = {}


def get_model(with_bias=False, legalize=True, iters=1, parts="pas"):
    key = (with_bias, legalize, iters, parts, SCORES_BD)
    if key not in _MODEL_CACHE:
        nc = _build_model(with_bias, iters=iters, parts=parts)
        if legalize:
            _legalize_sync(nc)
        _MODEL_CACHE[key] = nc
    return _MODEL_CACHE[key]


def make_in_maps(x, Wq, bq, Wk, bk, Wv, bv, Bbias):
    """Host-side sharding + layout prep. Returns (in_maps, with_bias)."""
    x = np.asarray(x, np.float32)
    with_bias = bool(np.any(bq) or np.any(bk))
    if np.any(bv):
        raise NotImplementedError("nonzero bv not supported")
    wq16 = np.ascontiguousarray(np.asarray(Wq, np.float32).T / 8.0).astype(np.float16)
    wk16 = np.ascontiguousarray(np.asarray(Wk, np.float32).T).astype(np.float16)
    wv16 = np.ascontiguousarray(np.asarray(Wv, np.float32).T).astype(np.float16)
    eb = np.exp(np.asarray(Bbias, np.float32).T)
    ebt = np.concatenate([eb, eb], 0).astype(np.float16)  # [128 (k x2), 64 q]
    common = {"wq": wq16, "wk": wk16, "wv": wv16, "ebt": ebt}
    if with_bias:
        bqk = np.concatenate(
            [np.asarray(bq, np.float32).reshape(4, 128).T / 8.0,
             np.asarray(bk, np.float32).reshape(4, 128).T], 1)  # [128, 8]
        common["bqk"] = np.ascontiguousarray(bqk)
    in_maps = []
    for b in range(B):
        xT16 = np.ascontiguousarray(
            x[b].reshape(TOK, C).T).astype(np.float16)
        in_maps.append({"xT": xT16, **common})
    return in_maps, with_bias


def kernel(**inputs):
    from concourse.bass_utils import run_bass_kernel_spmd
    in_maps, with_bias = make_in_maps(**inputs)
    nc = get_model(with_bias)
    res = run_bass_kernel_spmd(
        nc, in_maps, core_ids=list(range(B)), trace=TRACE)
    LAST["results"] = res
    out = np.stack([r["out"] for r in res.results], 0)
    return out.reshape(B, C, HH, WW)


def _harvest_io(nc):
    import jax
    in_names, out_names, out_avals = [], [], []
    for alloc in nc.m.functions[0].allocations:
        if not isinstance(alloc, mybir.MemoryLocationSet):
            continue
        name = alloc.memorylocations[0].name
        if alloc.kind == "ExternalInput":
            in_names.append(name)
        elif alloc.kind == "ExternalOutput":
            out_names.append(name)
            out_avals.append(jax.core.ShapedArray(
                tuple(alloc.tensor_shape), mybir.dt.np(alloc.dtype)))
    return in_names, out_names, out_avals


def _make_timed_callable(nc, in_maps):
    """Build a jitted shard_map callable around the single bass_exec of
    `nc` (mirrors run_bass_via_pjrt, but with NO donation so the same
    device-resident args can be reused across timed calls; outputs are
    garbage — timing only). Returns a zero-arg closure that runs one
    dispatch and blocks."""
    import jax
    from jax.sharding import Mesh, PartitionSpec
    from jax.experimental.shard_map import shard_map
    from concourse import bass2jax

    bass2jax.install_neuronx_cc_hook()
    in_names, out_names, out_avals = _harvest_io(nc)
    n_params = len(in_names)
    all_names = tuple(in_names + out_names)
    n_cores = len(in_maps)

    def _body(*args):
        return tuple(bass2jax._bass_exec_p.bind(
            *args,
            out_avals=tuple(out_avals),
            in_names=all_names,
            out_names=tuple(out_names),
            lowering_input_output_aliases=(),
            sim_require_finite=True,
            sim_require_nnan=True,
            nc=nc))

    devices = jax.devices()[:n_cores]
    mesh = Mesh(np.asarray(devices), ("core",))
    n_all = n_params + len(out_names)
    sharded = jax.jit(shard_map(
        _body, mesh=mesh,
        in_specs=(PartitionSpec("core"),) * n_all,
        out_specs=(PartitionSpec("core"),) * len(out_names),
        check_rep=False), keep_unused=True)
    concat_in = [
        np.concatenate([np.asarray(m[name]) for m in in_maps], 0)
        for name in in_names]
    concat_zeros = [
        np.zeros((n_cores * a.shape[0], *a.shape[1:]), a.dtype)
        for a in out_avals]
    args = [jax.device_put(a) for a in concat_in + concat_zeros]
    jax.block_until_ready(sharded(*args))  # warm-up / compile

    def run():
        jax.block_until_ready(sharded(*args))
    return run


def time_kernel(inputs, iters=4096, samples=8, parts="pas"):
    """Returns ns per iteration. Builds two model variants — the body run
    once vs `1+iters` times inside an on-device For_i loop — and
    differences median wall-clock over `samples` dispatches of each. With
    ~1s on-device per N-iter dispatch, the ~±20ms axon dispatch jitter
    contributes <2% error."""
    import time
    in_maps, with_bias = make_in_maps(**inputs)
    run1 = _make_timed_callable(
        get_model(with_bias, iters=1, parts=parts), in_maps)
    runN = _make_timed_callable(
        get_model(with_bias, iters=1 + iters, parts=parts), in_maps)
    t1s, tNs = [], []
    for _ in range(samples):
        t0 = time.time(); run1(); t1s.append(time.time() - t0)
        t0 = time.time(); runN(); tNs.append(time.time() - t0)
    t1 = float(np.median(t1s)); tN = float(np.median(tNs))
    return (tN - t1) / iters * 1e9, (t1s, tNs)



# revision 52
# speedup vs baseline: 3.9338x; 1.0006x over previous
"""Windowed multi-head attention TRN2 kernel (Bass/Tile), SPMD over 8 cores.

Problem (per reference): x:(8,512,64,64) viewed as (B, 4096 tok, 512 c);
Q/K/V = tok @ W^T + b; per window (64 tok) & head (8 x 64d):
softmax(QK^T/8 + Bbias) @ V; output back in (B,512,64,64).

Sharding: data-parallel, one batch element per core (8 cores).

Per-core dataflow (all matmuls fp16 operands, fp32 PSUM accum):
 - host passes x^T (c, tok) fp16 so projection rhs tiles DMA contiguously
 - Q^T,K^T computed in [c_out, tok] layout (heads pairs on partition halves)
 - V computed in natural [tok, c] layout, with a per-head ones-column
   appended (65-wide head blocks) so PV matmuls also produce softmax
   denominators
 - scores^T = K^T_wh^T-matmul: [k,q] tiles packed 8 units/PSUM bank,
   head-parity (e) on partition halves
 - softmax without max-subtraction (scores are O(1)): exp on ACT; the
   exp(Bbias^T) elementwise multiply on DVE simultaneously moves probs
   to window-parity (p) partition halves, so PV runs against natural V
   (no duplicate) and outputs land in natural token rows
 - PV: [64q, 65] units, 4 per PSUM bank; normalize via one batched
   reciprocal + one batched multiply per bank during PSUM->SBUF evac
"""

import sys
import numpy as np

for _p in ("/opt/trn_rl_repo",):
    if _p not in sys.path:
        sys.path.insert(0, _p)

from contextlib import ExitStack

import concourse.bass as bass
import concourse.tile as tile
from concourse import mybir

F16 = mybir.dt.float16
F32 = mybir.dt.float32

B, C, HH, WW = 8, 512, 64, 64
NH, HD = 8, 64
WIN = 64            # tokens per window
TOK = C * 0 + 4096  # tokens per batch/core
NT = 8              # 512-token tiles per core
NCHUNK = 4          # 128-channel chunks

TRACE = False
LAST = {}
SCORES_BD = True  # scores via block-diagonal K (full 128-part contraction)
PIPELINE = 1      # emit attention this many T-tiles behind projections
STORE_SP = True   # issue output stores from SP instead of ACT
PROJBUFS = 3      # PSUM banks for projection groups
OPSBUFS = 1       # PSUM buffers per PV output tag (2 tags)
EBUFS = 2         # SBUF buffers for attention et/pt/on tiles
INTERLEAVE = 1    # emit attention subtiles between projection groups


def _emit(tc, out, xT, wq, wk, wv, ebt, bqk, iters=1, parts="pas"):
    """Emit the per-core program. bqk: [128, 8] fp32 (bq/8 | bk chunks) or None.
    parts: subset of 'p' (projections), 'a' (attention), 's' (store) for
    timing ablations."""
    nc = tc.nc
    Exp = mybir.ActivationFunctionType.Exp
    Ident = mybir.ActivationFunctionType.Identity

    with ExitStack() as ctx:
        ep = ctx.enter_context

        tbufs = PIPELINE + 1
        wpool = ep(tc.tile_pool(name="w", bufs=1))
        xpool = ep(tc.tile_pool(name="x", bufs=tbufs))
        qkpool = ep(tc.tile_pool(name="qk", bufs=tbufs))
        vpool = ep(tc.tile_pool(name="v", bufs=tbufs))
        epool = ep(tc.tile_pool(name="e", bufs=EBUFS))
        bdpool = ep(tc.tile_pool(name="bd", bufs=tbufs))
        rcpool = ep(tc.tile_pool(name="rc", bufs=4))
        onpool = ep(tc.tile_pool(name="on", bufs=EBUFS))
        projps = ep(tc.tile_pool(name="projps", bufs=PROJBUFS, space="PSUM"))
        sps = ep(tc.tile_pool(name="sps", bufs=2, space="PSUM"))
        ops = ep(tc.tile_pool(name="ops", bufs=OPSBUFS, space="PSUM"))

        # resident weights: [c_in chunk 128, c_out 512] fp16 per proj
        wsb = {}
        for nm, wdram in (("q", wq), ("k", wk), ("v", wv)):
            for ci in range(NCHUNK):
                t = wpool.tile([128, 512], F16, tag=f"w{nm}{ci}")
                nc.sync.dma_start(t[:], wdram[ci * 128:(ci + 1) * 128, :])
                wsb[nm, ci] = t
        ebt_sb = wpool.tile([128, 64], F16, tag="ebt")
        nc.sync.dma_start(ebt_sb[:], ebt[:, :])
        bqk_sb = None
        if bqk is not None:
            bqk_sb = wpool.tile([128, 8], F32, tag="bqk")
            nc.sync.dma_start(bqk_sb[:], bqk[:, :])

        # ---- one-time inits: zero blocks of block-diagonal tiles and the
        # ones-columns of V survive every iteration (later writes only
        # touch the data blocks), so initialize all rotating buffers here,
        # outside the timing loop. Tile calls advance each tag's rotation
        # by bufs=2, preserving in-loop phase.
        for i in range(max(tbufs, EBUFS)):
            if SCORES_BD and i < tbufs:
                for j in range(4):
                    t = bdpool.tile([128, 1024], F16, tag=f"bdk{j}")
                    nc.gpsimd.memset(t[:], 0)
            if i < EBUFS:
                t = epool.tile([128, 1024], F16, tag="pt")
                nc.gpsimd.memset(t[:], 0)
            if i < tbufs:
                for tt in range(NCHUNK):
                    vn = vpool.tile([128, 520], F16, tag=f"vn{tt}")
                    nc.scalar.activation(
                        vn[:].rearrange("p (h x) -> p h x", x=65)[:, :, 64],
                        ebt_sb[:, 0:8], Ident, bias=1.0, scale=0.0)

        if iters > 1:
            # on-device repetition for timing: amortizes host dispatch
            ep(tc.For_i(0, iters))

        def emit_proj_group(nm, co, xt, dst, bdk=None):
            """One projection PSUM group (4 matmuls) + its evacuation."""
            pi = 0 if nm == "q" else 1
            ps = projps.tile([128, 512], F32, tag="proj")
            if nm == "v":
                for ci in range(NCHUNK):
                    nc.tensor.matmul(
                        ps[:],
                        xt[ci][:, co * 128:(co + 1) * 128],
                        wsb["v", ci][:],
                        start=(ci == 0), stop=(ci == NCHUNK - 1))
                if "P" not in parts:
                    nc.vector.tensor_copy(
                        dst[:].rearrange("p (h x) -> p h x", x=65)
                        [:, :, 0:64],
                        ps[:].rearrange("p (h x) -> p h x", x=64))
                return
            for ci in range(NCHUNK):
                w_ap = (wsb[nm, 0][:, 0:128] if "W" in parts
                        else wsb[nm, ci][:, co * 128:(co + 1) * 128])
                nc.tensor.matmul(
                    ps[:], w_ap, xt[0 if "W" in parts else ci][:],
                    start=(ci == 0), stop=(ci == NCHUNK - 1))
            if "P" in parts:
                return
            if bdk is not None:
                # K straight into block-diagonal layout, window-major:
                # col g*128 + e*64 + k, g = 2tt+p
                bd_v = bdk[:].rearrange("r (g c) -> r g c", c=128)
                ps_v = ps[:].rearrange("r (g k) -> r g k", k=64)
                for e in range(2):
                    re = slice(e * 64, e * 64 + 64)
                    if bqk_sb is not None:
                        nc.scalar.activation(
                            bd_v[re, :, e * 64:e * 64 + 64],
                            ps_v[re], Ident,
                            bias=bqk_sb[re, 4 + co:5 + co])
                    else:
                        nc.scalar.copy(
                            bd_v[re, :, e * 64:e * 64 + 64], ps_v[re])
            elif bqk_sb is not None:
                nc.scalar.activation(
                    dst[:], ps[:], Ident,
                    bias=bqk_sb[:, pi * 4 + co:pi * 4 + co + 1])
            else:
                nc.scalar.copy(dst[:], ps[:])

        def proj_thunks(T):
            """xt loads (immediate) + 12 emission thunks for T's
            projection PSUM groups; returns (thunks, state_entry)."""
            xt = []
            for ci in range(NCHUNK):
                t = xpool.tile([128, 512], F16, tag=f"xt{ci}")
                nc.sync.dma_start(
                    t[:],
                    xT[ci * 128:(ci + 1) * 128, T * 512:(T + 1) * 512])
                xt.append(t)
            qkt = {}
            bdks = []
            vnat = []
            thunks = []
            names = ("q",) if SCORES_BD else ("q", "k")
            for nm in names:
                for co in range(NCHUNK):
                    t = qkpool.tile([128, 512], F16, tag=f"{nm}t{co}")
                    qkt[nm, co] = t
                    if "p" in parts:
                        thunks.append(
                            lambda nm=nm, co=co, t=t: emit_proj_group(
                                nm, co, xt, t))
            if SCORES_BD:
                for j in range(4):
                    bdk = bdpool.tile([128, 1024], F16, tag=f"bdk{j}")
                    bdks.append(bdk)
                    if "p" in parts:
                        thunks.append(
                            lambda j=j, bdk=bdk: emit_proj_group(
                                "k", j, xt, None, bdk=bdk))
            for tt in range(NCHUNK):
                vn = vpool.tile([128, 520], F16, tag=f"vn{tt}")
                vnat.append(vn)
                if "p" in parts:
                    thunks.append(
                        lambda tt=tt, vn=vn: emit_proj_group(
                            "v", tt, xt, vn))
            return thunks, (qkt, bdks, vnat)

        def emit_attn_scores(qkt, bdks, Ta, tt):
            # ---- attention: subtile tt covers windows 2tt, 2tt+1 of Ta.
            # HAZARD RULE: concurrent matmuls with disjoint row-groups but
            # a shared column-group collide in the PE array (device crash);
            # sub-128 matmuls are placed DIAGONALLY (out partition base ==
            # operand partition base). Scores land head-parity packed (e on
            # halves); the exp(Bbias)-multiply on DVE moves probs to
            # block-diagonal window-parity layout, so PV runs full-width
            # against natural V and outputs land in natural token rows.
            if True:
                s = sps.tile([128, 512], F32, tag="s")
                if SCORES_BD:
                    for j in range(4):
                        for p in range(2):
                            w = 2 * tt + p
                            nc.tensor.matmul(
                                s[:, (j * 2 + p) * 64:(j * 2 + p + 1) * 64],
                                bdks[j][:, tt * 256 + p * 128:
                                        tt * 256 + (p + 1) * 128],
                                qkt["q", j][:, w * 64:(w + 1) * 64],
                                start=True, stop=True)
                else:
                    for j in range(4):
                        for e in range(2):
                            r = slice(e * 64, e * 64 + 64)
                            for p in range(2):
                                w = 2 * tt + p
                                wc = slice(w * 64, w * 64 + 64)
                                nc.tensor.matmul(
                                    s[r, (j * 2 + p) * 64:
                                      (j * 2 + p + 1) * 64],
                                    qkt["k", j][r, wc],
                                    qkt["q", j][r, wc],
                                    start=True, stop=True)
                et = epool.tile([128, 512], F16, tag="et")
                nc.scalar.activation(et[:], s[:], Exp)
                # block-diagonal probs: pt[p*64+k, h*128+p*64+q] =
                # et[e*64+k, (2j+p)*64+q]*ebt[k,q] (h=2j+e); off-diagonal
                # blocks stay zero, so one PV matmul covers both windows
                # with full 128-partition contraction against natural V.
                pt = epool.tile([128, 1024], F16, tag="pt")
                et_v = et[:].rearrange("r (j u q) -> r j u q", u=2, q=64)
                pt_v = pt[:].rearrange("r (j z) -> r j z", j=4)
                for p in range(2):
                    rp = slice(p * 64, p * 64 + 64)
                    for e in range(2):
                        re = slice(e * 64, e * 64 + 64)
                        c0 = e * 128 + p * 64
                        nc.vector.tensor_mul(
                            pt_v[rp, :, c0:c0 + 64],
                            et_v[re, :, p, :],
                            ebt_sb[re, 0:64].unsqueeze(1)
                            .broadcast_to((64, 4, 64)))
                return pt

        def emit_attn_pv(pt, vnat, Ta, tt):
            if True:
                # PV: 8 matmuls (one per head), full 128 partitions; two
                # PSUM banks of 4 [128q2w, 65] units each.
                on = onpool.tile([128, 512], F32, tag=f"on{tt % 2}")
                for b in range(2):
                    o = ops.tile([128, 260], F32, tag=f"ob{b}")
                    o_v = o[:].rearrange("r (u x) -> r u x", x=65)
                    for u in range(4):
                        h = 4 * b + u
                        nc.tensor.matmul(
                            o[:, u * 65:(u + 1) * 65],
                            pt[:, h * 128:(h + 1) * 128],
                            vnat[tt][:, h * 65:(h + 1) * 65],
                            start=True, stop=True)
                    rc = rcpool.tile([128, 4], F32, tag=f"rc{b}")
                    nc.vector.reciprocal(rc[:, 0:4], o_v[:, :, 64])
                    nc.vector.tensor_mul(
                        on[:].rearrange("r (b2 u q) -> r b2 u q", b2=2, q=64)
                        [:, b, :, :],
                        o_v[:, :, 0:64],
                        rc[:, 0:4].unsqueeze(2).broadcast_to((128, 4, 64)))
                if "s" in parts:
                    eng = nc.sync if STORE_SP else nc.scalar
                    eng.dma_start(
                        out[Ta * 512 + tt * 128: Ta * 512 + (tt + 1) * 128,
                            :],
                        on[:])

        # software pipeline driver: attention trails projections by
        # PIPELINE T-tiles; with INTERLEAVE, attention subtiles are emitted
        # between projection groups as scheduler priority hints.
        state = {}
        for T in range(NT + PIPELINE):
            pthunks = []
            if T < NT:
                pthunks, entry = proj_thunks(T)
                state[T] = entry
            athunks = []
            if T >= PIPELINE and "a" in parts:
                Ta = T - PIPELINE
                q_, b_, v_ = state.pop(Ta)
                for tt in range(NCHUNK):
                    cell = {}

                    def a_sc(tt=tt, q=q_, bb=b_, Ta=Ta, cell=cell):
                        cell["pt"] = emit_attn_scores(q, bb, Ta, tt)

                    def a_pv(tt=tt, v=v_, Ta=Ta, cell=cell):
                        emit_attn_pv(cell["pt"], v, Ta, tt)

                    if INTERLEAVE == 2:
                        athunks += [a_sc, a_pv]
                    else:
                        athunks.append(lambda a=a_sc, b=a_pv: (a(), b()))
            if INTERLEAVE and pthunks and athunks:
                for i, th in enumerate(pthunks):
                    th()
                    if INTERLEAVE == 2:
                        if i % 3 != 0 and athunks:
                            athunks.pop(0)()
                    elif i % 3 == 2 and athunks:
                        athunks.pop(0)()
            else:
                for th in pthunks:
                    th()
            for th in athunks:
                th()


def _legalize_sync(nc, max_waits=1):
    """Hoist excess semaphore waits into standalone same-engine
    EventSemaphore instructions. Engine instruction streams execute in
    order, so a wait carried by an immediately-preceding EventSemaphore is
    equivalent to a wait on the instruction itself — and the walrus build
    in this environment rejects instructions with more than one wait."""
    import bass_rust
    n_new = 0
    fn = nc.m.functions[0]
    for blk in fn.blocks:
        out = []
        changed = False
        for ins in blk.instructions:
            si = ins.sync_info
            waits = list(si.on_wait) if si and si.on_wait else []
            if len(waits) > max_waits:
                keep = waits[-max_waits:]
                for w in waits[:-max_waits]:
                    es = mybir.InstEventSemaphore(
                        name=f"esw-{n_new}-{ins.name}", ins=[], outs=[])
                    es.engine = ins.engine
                    es.sync_info = bass_rust.SyncInfo(on_wait=[w], on_update=[])
                    out.append(es)
                    n_new += 1
                ins.sync_info = bass_rust.SyncInfo(
                    on_wait=keep,
                    on_update=list(si.on_update) if si.on_update else [])
                changed = True
            out.append(ins)
        if changed:
            blk.instructions = out
    return n_new


def _build_model(with_bias, iters=1, parts="pas"):
    nc = bass.Bass("TRN2", target_bir_lowering=False, debug=False,
                   enable_partition_id=False)
    xT = nc.dram_tensor("xT", [512, 4096], F16, kind="ExternalInput").ap()
    wq = nc.dram_tensor("wq", [512, 512], F16, kind="ExternalInput").ap()
    wk = nc.dram_tensor("wk", [512, 512], F16, kind="ExternalInput").ap()
    wv = nc.dram_tensor("wv", [512, 512], F16, kind="ExternalInput").ap()
    ebt = nc.dram_tensor("ebt", [128, 64], F16, kind="ExternalInput").ap()
    bqk = (nc.dram_tensor("bqk", [128, 8], F32, kind="ExternalInput").ap()
           if with_bias else None)
    out = nc.dram_tensor("out", [4096, 512], F32, kind="ExternalOutput").ap()
    with tile.TileContext(nc) as tc:
        _emit(tc, out, xT, wq, wk, wv, ebt, bqk, iters=iters, parts=parts)
    return nc


_MODEL_CACHE = {}


def get_model(with_bias=False, legalize=True, iters=1, parts="pas"):
    key = (with_bias, legalize, iters, parts, SCORES_BD, PIPELINE, STORE_SP,
           PROJBUFS, OPSBUFS, EBUFS, INTERLEAVE)
    if key not in _MODEL_CACHE:
        nc = _build_model(with_bias, iters=iters, parts=parts)
        if legalize:
            _legalize_sync(nc)
        _MODEL_CACHE[key] = nc
    return _MODEL_CACHE[key]


def make_in_maps(x, Wq, bq, Wk, bk, Wv, bv, Bbias):
    """Host-side sharding + layout prep. Returns (in_maps, with_bias)."""
    x = np.asarray(x, np.float32)
    with_bias = bool(np.any(bq) or np.any(bk))
    if np.any(bv):
        raise NotImplementedError("nonzero bv not supported")
    wq16 = np.ascontiguousarray(np.asarray(Wq, np.float32).T / 8.0).astype(np.float16)
    wk16 = np.ascontiguousarray(np.asarray(Wk, np.float32).T).astype(np.float16)
    wv16 = np.ascontiguousarray(np.asarray(Wv, np.float32).T).astype(np.float16)
    eb = np.exp(np.asarray(Bbias, np.float32).T)
    ebt = np.concatenate([eb, eb], 0).astype(np.float16)  # [128 (k x2), 64 q]
    common = {"wq": wq16, "wk": wk16, "wv": wv16, "ebt": ebt}
    if with_bias:
        bqk = np.concatenate(
            [np.asarray(bq, np.float32).reshape(4, 128).T / 8.0,
             np.asarray(bk, np.float32).reshape(4, 128).T], 1)  # [128, 8]
        common["bqk"] = np.ascontiguousarray(bqk)
    in_maps = []
    for b in range(B):
        xT16 = np.ascontiguousarray(
            x[b].reshape(TOK, C).T).astype(np.float16)
        in_maps.append({"xT": xT16, **common})
    return in_maps, with_bias


def kernel(**inputs):
    from concourse.bass_utils import run_bass_kernel_spmd
    in_maps, with_bias = make_in_maps(**inputs)
    nc = get_model(with_bias)
    res = run_bass_kernel_spmd(
        nc, in_maps, core_ids=list(range(B)), trace=TRACE)
    LAST["results"] = res
    out = np.stack([r["out"] for r in res.results], 0)
    return out.reshape(B, C, HH, WW)


def _harvest_io(nc):
    import jax
    in_names, out_names, out_avals = [], [], []
    for alloc in nc.m.functions[0].allocations:
        if not isinstance(alloc, mybir.MemoryLocationSet):
            continue
        name = alloc.memorylocations[0].name
        if alloc.kind == "ExternalInput":
            in_names.append(name)
        elif alloc.kind == "ExternalOutput":
            out_names.append(name)
            out_avals.append(jax.core.ShapedArray(
                tuple(alloc.tensor_shape), mybir.dt.np(alloc.dtype)))
    return in_names, out_names, out_avals


def _make_timed_callable(nc, in_maps):
    """Build a jitted shard_map callable around the single bass_exec of
    `nc` (mirrors run_bass_via_pjrt, but with NO donation so the same
    device-resident args can be reused across timed calls; outputs are
    garbage — timing only). Returns a zero-arg closure that runs one
    dispatch and blocks."""
    import jax
    from jax.sharding import Mesh, PartitionSpec
    from jax.experimental.shard_map import shard_map
    from concourse import bass2jax

    bass2jax.install_neuronx_cc_hook()
    in_names, out_names, out_avals = _harvest_io(nc)
    n_params = len(in_names)
    all_names = tuple(in_names + out_names)
    n_cores = len(in_maps)

    def _body(*args):
        return tuple(bass2jax._bass_exec_p.bind(
            *args,
            out_avals=tuple(out_avals),
            in_names=all_names,
            out_names=tuple(out_names),
            lowering_input_output_aliases=(),
            sim_require_finite=True,
            sim_require_nnan=True,
            nc=nc))

    devices = jax.devices()[:n_cores]
    mesh = Mesh(np.asarray(devices), ("core",))
    n_all = n_params + len(out_names)
    sharded = jax.jit(shard_map(
        _body, mesh=mesh,
        in_specs=(PartitionSpec("core"),) * n_all,
        out_specs=(PartitionSpec("core"),) * len(out_names),
        check_rep=False), keep_unused=True)
    concat_in = [
        np.concatenate([np.asarray(m[name]) for m in in_maps], 0)
        for name in in_names]
    concat_zeros = [
        np.zeros((n_cores * a.shape[0], *a.shape[1:]), a.dtype)
        for a in out_avals]
    args = [jax.device_put(a) for a in concat_in + concat_zeros]
    jax.block_until_ready(sharded(*args))  # warm-up / compile

    def run():
        jax.block_until_ready(sharded(*args))
    return run


def time_kernel(inputs, iters=4096, samples=8, parts="pas"):
    """Returns ns per iteration. Builds two model variants — the body run
    once vs `1+iters` times inside an on-device For_i loop — and
    differences median wall-clock over `samples` dispatches of each. With
    ~1s on-device per N-iter dispatch, the ~±20ms axon dispatch jitter
    contributes <2% error."""
    import time
    in_maps, with_bias = make_in_maps(**inputs)
    run1 = _make_timed_callable(
        get_model(with_bias, iters=1, parts=parts), in_maps)
    runN = _make_timed_callable(
        get_model(with_bias, iters=1 + iters, parts=parts), in_maps)
    t1s, tNs = [], []
    for _ in range(samples):
        t0 = time.time(); run1(); t1s.append(time.time() - t0)
        t0 = time.time(); runN(); tNs.append(time.time() - t0)
    t1 = float(np.median(t1s)); tN = float(np.median(tNs))
    return (tN - t1) / iters * 1e9, (t1s, tNs)



# revision 66
# speedup vs baseline: 4.0483x; 1.0291x over previous
"""Windowed multi-head attention TRN2 kernel (Bass/Tile), SPMD over 8 cores.

Problem (per reference): x:(8,512,64,64) viewed as (B, 4096 tok, 512 c);
Q/K/V = tok @ W^T + b; per window (64 tok) & head (8 x 64d):
softmax(QK^T/8 + Bbias) @ V; output back in (B,512,64,64).

Sharding: data-parallel, one batch element per core (8 cores).

Per-core dataflow (all matmuls fp16 operands, fp32 PSUM accum):
 - host passes x^T (c, tok) fp16 so projection rhs tiles DMA contiguously
 - Q^T,K^T computed in [c_out, tok] layout (heads pairs on partition halves)
 - V computed in natural [tok, c] layout, with a per-head ones-column
   appended (65-wide head blocks) so PV matmuls also produce softmax
   denominators
 - scores^T = K^T_wh^T-matmul: [k,q] tiles packed 8 units/PSUM bank,
   head-parity (e) on partition halves
 - softmax without max-subtraction (scores are O(1)): exp on ACT; the
   exp(Bbias^T) elementwise multiply on DVE simultaneously moves probs
   to window-parity (p) partition halves, so PV runs against natural V
   (no duplicate) and outputs land in natural token rows
 - PV: [64q, 65] units, 4 per PSUM bank; normalize via one batched
   reciprocal + one batched multiply per bank during PSUM->SBUF evac
"""

import sys
import numpy as np

for _p in ("/opt/trn_rl_repo",):
    if _p not in sys.path:
        sys.path.insert(0, _p)

from contextlib import ExitStack

import concourse.bass as bass
import concourse.tile as tile
from concourse import mybir

F16 = mybir.dt.float16
F32 = mybir.dt.float32

B, C, HH, WW = 8, 512, 64, 64
NH, HD = 8, 64
WIN = 64            # tokens per window
TOK = C * 0 + 4096  # tokens per batch/core
NT = 8              # 512-token tiles per core
NCHUNK = 4          # 128-channel chunks

TRACE = False
LAST = {}
SCORES_BD = True  # scores via block-diagonal K (full 128-part contraction)
PIPELINE = 1      # emit attention this many T-tiles behind projections
STORE_SP = True   # issue output stores from SP instead of ACT
PROJBUFS = 3      # PSUM banks for projection groups
OPSBUFS = 1       # PSUM buffers per PV output tag (2 tags)
SPSBUFS = 2       # PSUM buffers for scores
ATTSPLIT = 1      # split exp/prob-mul for finer attention wavefront
PMAJOR = 1        # scores matmuls in window-parity-major order
EBUFS = 2         # SBUF buffers for attention et/pt/on tiles
INTERLEAVE = 1    # emit attention subtiles between projection groups


def _emit(tc, out, xT, wq, wk, wv, ebt, bqk, iters=1, parts="pas"):
    """Emit the per-core program. bqk: [128, 8] fp32 (bq/8 | bk chunks) or None.
    parts: subset of 'p' (projections), 'a' (attention), 's' (store) for
    timing ablations."""
    nc = tc.nc
    Exp = mybir.ActivationFunctionType.Exp
    Ident = mybir.ActivationFunctionType.Identity

    with ExitStack() as ctx:
        ep = ctx.enter_context

        tbufs = PIPELINE + 1
        wpool = ep(tc.tile_pool(name="w", bufs=1))
        xpool = ep(tc.tile_pool(name="x", bufs=tbufs))
        qkpool = ep(tc.tile_pool(name="qk", bufs=tbufs))
        vpool = ep(tc.tile_pool(name="v", bufs=tbufs))
        epool = ep(tc.tile_pool(name="e", bufs=EBUFS))
        bdpool = ep(tc.tile_pool(name="bd", bufs=tbufs))
        rcpool = ep(tc.tile_pool(name="rc", bufs=4))
        onpool = ep(tc.tile_pool(name="on", bufs=EBUFS))
        projps = ep(tc.tile_pool(name="projps", bufs=PROJBUFS, space="PSUM"))
        sps = ep(tc.tile_pool(name="sps", bufs=SPSBUFS, space="PSUM"))
        ops = ep(tc.tile_pool(name="ops", bufs=OPSBUFS, space="PSUM"))

        # resident weights: [c_in chunk 128, c_out 512] fp16 per proj
        wsb = {}
        for nm, wdram in (("q", wq), ("k", wk), ("v", wv)):
            for ci in range(NCHUNK):
                t = wpool.tile([128, 512], F16, tag=f"w{nm}{ci}")
                nc.sync.dma_start(t[:], wdram[ci * 128:(ci + 1) * 128, :])
                wsb[nm, ci] = t
        ebt_sb = wpool.tile([128, 64], F16, tag="ebt")
        nc.sync.dma_start(ebt_sb[:], ebt[:, :])
        bqk_sb = None
        if bqk is not None:
            bqk_sb = wpool.tile([128, 8], F32, tag="bqk")
            nc.sync.dma_start(bqk_sb[:], bqk[:, :])

        # ---- one-time inits: zero blocks of block-diagonal tiles and the
        # ones-columns of V survive every iteration (later writes only
        # touch the data blocks), so initialize all rotating buffers here,
        # outside the timing loop. Tile calls advance each tag's rotation
        # by bufs=2, preserving in-loop phase.
        for i in range(max(tbufs, EBUFS)):
            if SCORES_BD and i < tbufs:
                for j in range(4):
                    t = bdpool.tile([128, 1024], F16, tag=f"bdk{j}")
                    nc.gpsimd.memset(t[:], 0)
            if i < EBUFS:
                t = epool.tile([128, 1024], F16, tag="pt")
                nc.gpsimd.memset(t[:], 0)
            if i < tbufs:
                for tt in range(NCHUNK):
                    vn = vpool.tile([128, 520], F16, tag=f"vn{tt}")
                    nc.scalar.activation(
                        vn[:].rearrange("p (h x) -> p h x", x=65)[:, :, 64],
                        ebt_sb[:, 0:8], Ident, bias=1.0, scale=0.0)

        if iters > 1:
            # on-device repetition for timing: amortizes host dispatch
            ep(tc.For_i(0, iters))

        def emit_proj_group(nm, co, xt, dst, bdk=None):
            """One projection PSUM group (4 matmuls) + its evacuation."""
            pi = 0 if nm == "q" else 1
            ps = projps.tile([128, 512], F32, tag="proj")
            if nm == "v":
                for ci in range(NCHUNK):
                    nc.tensor.matmul(
                        ps[:],
                        xt[ci][:, co * 128:(co + 1) * 128],
                        wsb["v", ci][:],
                        start=(ci == 0), stop=(ci == NCHUNK - 1))
                if "P" not in parts:
                    nc.vector.tensor_copy(
                        dst[:].rearrange("p (h x) -> p h x", x=65)
                        [:, :, 0:64],
                        ps[:].rearrange("p (h x) -> p h x", x=64))
                return
            for ci in range(NCHUNK):
                w_ap = (wsb[nm, 0][:, 0:128] if "W" in parts
                        else wsb[nm, ci][:, co * 128:(co + 1) * 128])
                if "A" in parts:  # ablation: independent matmuls, no accum
                    nc.tensor.matmul(
                        ps[:], w_ap, xt[0 if "W" in parts else ci][:],
                        start=True, stop=True)
                else:
                    nc.tensor.matmul(
                        ps[:], w_ap, xt[0 if "W" in parts else ci][:],
                        start=(ci == 0), stop=(ci == NCHUNK - 1))
            if "P" in parts:
                return
            if bdk is not None:
                # K straight into block-diagonal layout, window-major:
                # col g*128 + e*64 + k, g = 2tt+p
                bd_v = bdk[:].rearrange("r (g c) -> r g c", c=128)
                ps_v = ps[:].rearrange("r (g k) -> r g k", k=64)
                for e in range(2):
                    re = slice(e * 64, e * 64 + 64)
                    if bqk_sb is not None:
                        nc.scalar.activation(
                            bd_v[re, :, e * 64:e * 64 + 64],
                            ps_v[re], Ident,
                            bias=bqk_sb[re, 4 + co:5 + co])
                    else:
                        nc.scalar.copy(
                            bd_v[re, :, e * 64:e * 64 + 64], ps_v[re])
            elif bqk_sb is not None:
                nc.scalar.activation(
                    dst[:], ps[:], Ident,
                    bias=bqk_sb[:, pi * 4 + co:pi * 4 + co + 1])
            else:
                nc.scalar.copy(dst[:], ps[:])

        def proj_thunks(T):
            """xt loads (immediate) + 12 emission thunks for T's
            projection PSUM groups; returns (thunks, state_entry)."""
            xt = []
            for ci in range(NCHUNK):
                t = xpool.tile([128, 512], F16, tag=f"xt{ci}")
                nc.sync.dma_start(
                    t[:],
                    xT[ci * 128:(ci + 1) * 128, T * 512:(T + 1) * 512])
                xt.append(t)
            qkt = {}
            bdks = []
            vnat = []
            thunks = []

            def emit_proj_pair(nm, co0, xt):
                # ablation: interleave two groups' accumulation chains so
                # consecutive matmuls never hit the same PSUM region
                ps0 = projps.tile([128, 512], F32, tag="proj")
                ps1 = projps.tile([128, 512], F32, tag="proj")
                for ci in range(NCHUNK):
                    for k, ps in ((0, ps0), (1, ps1)):
                        co = co0 + k
                        nc.tensor.matmul(
                            ps[:],
                            wsb[nm, ci][:, co * 128:(co + 1) * 128],
                            xt[ci][:],
                            start=(ci == 0), stop=(ci == NCHUNK - 1))

            names = ("q",) if SCORES_BD else ("q", "k")
            for nm in names:
                if "I" in parts and "p" in parts:
                    for co0 in (0, 2):
                        thunks.append(
                            lambda nm=nm, co0=co0: emit_proj_pair(
                                nm, co0, xt))
                    for co in range(NCHUNK):
                        t = qkpool.tile([128, 512], F16, tag=f"{nm}t{co}")
                        qkt[nm, co] = t
                    continue
                for co in range(NCHUNK):
                    t = qkpool.tile([128, 512], F16, tag=f"{nm}t{co}")
                    qkt[nm, co] = t
                    if "p" in parts:
                        thunks.append(
                            lambda nm=nm, co=co, t=t: emit_proj_group(
                                nm, co, xt, t))
            if SCORES_BD:
                for j in range(4):
                    bdk = bdpool.tile([128, 1024], F16, tag=f"bdk{j}")
                    bdks.append(bdk)
                    if "p" in parts:
                        thunks.append(
                            lambda j=j, bdk=bdk: emit_proj_group(
                                "k", j, xt, None, bdk=bdk))
            for tt in range(NCHUNK):
                vn = vpool.tile([128, 520], F16, tag=f"vn{tt}")
                vnat.append(vn)
                if "p" in parts:
                    thunks.append(
                        lambda tt=tt, vn=vn: emit_proj_group(
                            "v", tt, xt, vn))
            return thunks, (qkt, bdks, vnat)

        def emit_attn_scores(qkt, bdks, Ta, tt):
            # ---- attention: subtile tt covers windows 2tt, 2tt+1 of Ta.
            # HAZARD RULE: concurrent matmuls with disjoint row-groups but
            # a shared column-group collide in the PE array (device crash);
            # sub-128 matmuls are placed DIAGONALLY (out partition base ==
            # operand partition base). Scores land head-parity packed (e on
            # halves); the exp(Bbias)-multiply on DVE moves probs to
            # block-diagonal window-parity layout, so PV runs full-width
            # against natural V and outputs land in natural token rows.
            if True:
                s = sps.tile([128, 512], F32, tag="s")
                if SCORES_BD:
                    # p-major: with ATTSPLIT, exp of parity p waits only on
                    # its own 4 matmuls
                    for p, j in (
                            [(p, j) for p in range(2) for j in range(4)]
                            if PMAJOR else
                            [(p, j) for j in range(4) for p in range(2)]):
                        if True:
                            w = 2 * tt + p
                            nc.tensor.matmul(
                                s[:, (j * 2 + p) * 64:(j * 2 + p + 1) * 64],
                                bdks[j][:, tt * 256 + p * 128:
                                        tt * 256 + (p + 1) * 128],
                                qkt["q", j][:, w * 64:(w + 1) * 64],
                                start=True, stop=True)
                else:
                    for j in range(4):
                        for e in range(2):
                            r = slice(e * 64, e * 64 + 64)
                            for p in range(2):
                                w = 2 * tt + p
                                wc = slice(w * 64, w * 64 + 64)
                                nc.tensor.matmul(
                                    s[r, (j * 2 + p) * 64:
                                      (j * 2 + p + 1) * 64],
                                    qkt["k", j][r, wc],
                                    qkt["q", j][r, wc],
                                    start=True, stop=True)
                et = epool.tile([128, 512], F16, tag="et")
                et_v = et[:].rearrange("r (j u q) -> r j u q", u=2, q=64)
                if ATTSPLIT:
                    # split exp per window parity: the (p, e) multiplies
                    # depend only on their own exp half
                    s_v = s[:].rearrange("r (j u q) -> r j u q", u=2, q=64)
                    for p in range(2):
                        nc.scalar.activation(
                            et_v[:, :, p, :], s_v[:, :, p, :], Exp)
                else:
                    nc.scalar.activation(et[:], s[:], Exp)
                # block-diagonal probs: pt[p*64+k, h*128+p*64+q] =
                # et[e*64+k, (2j+p)*64+q]*ebt[k,q] (h=2j+e); off-diagonal
                # blocks stay zero, so one PV matmul covers both windows
                # with full 128-partition contraction against natural V.
                pt = epool.tile([128, 1024], F16, tag="pt")
                pt_v = pt[:].rearrange("r (j z) -> r j z", j=4)
                for p in range(2):
                    rp = slice(p * 64, p * 64 + 64)
                    for e in range(2):
                        re = slice(e * 64, e * 64 + 64)
                        c0 = e * 128 + p * 64
                        if ATTSPLIT:
                            # split per bank-half: PV bank b waits only on
                            # the j in {2b, 2b+1} multiplies
                            for bh in range(2):
                                js = slice(bh * 2, bh * 2 + 2)
                                nc.vector.tensor_mul(
                                    pt_v[rp, js, c0:c0 + 64],
                                    et_v[re, js, p, :],
                                    ebt_sb[re, 0:64].unsqueeze(1)
                                    .broadcast_to((64, 2, 64)))
                        else:
                            nc.vector.tensor_mul(
                                pt_v[rp, :, c0:c0 + 64],
                                et_v[re, :, p, :],
                                ebt_sb[re, 0:64].unsqueeze(1)
                                .broadcast_to((64, 4, 64)))
                return pt

        def emit_attn_pv(pt, vnat, Ta, tt):
            if True:
                # PV: 8 matmuls (one per head), full 128 partitions; two
                # PSUM banks of 4 [128q2w, 65] units each.
                on = onpool.tile([128, 512], F32, tag=f"on{tt % 2}")
                for b in range(2):
                    o = ops.tile([128, 260], F32, tag=f"ob{b}")
                    o_v = o[:].rearrange("r (u x) -> r u x", x=65)
                    for u in range(4):
                        h = 4 * b + u
                        nc.tensor.matmul(
                            o[:, u * 65:(u + 1) * 65],
                            pt[:, h * 128:(h + 1) * 128],
                            vnat[tt][:, h * 65:(h + 1) * 65],
                            start=True, stop=True)
                    rc = rcpool.tile([128, 4], F32, tag=f"rc{b}")
                    nc.vector.reciprocal(rc[:, 0:4], o_v[:, :, 64])
                    nc.vector.tensor_mul(
                        on[:].rearrange("r (b2 u q) -> r b2 u q", b2=2, q=64)
                        [:, b, :, :],
                        o_v[:, :, 0:64],
                        rc[:, 0:4].unsqueeze(2).broadcast_to((128, 4, 64)))
                if "s" in parts:
                    eng = nc.sync if STORE_SP else nc.scalar
                    eng.dma_start(
                        out[Ta * 512 + tt * 128: Ta * 512 + (tt + 1) * 128,
                            :],
                        on[:])

        # software pipeline driver: attention trails projections by
        # PIPELINE T-tiles; with INTERLEAVE, attention subtiles are emitted
        # between projection groups as scheduler priority hints.
        state = {}
        for T in range(NT + PIPELINE):
            pthunks = []
            if T < NT:
                pthunks, entry = proj_thunks(T)
                state[T] = entry
            athunks = []
            if T >= PIPELINE and "a" in parts:
                Ta = T - PIPELINE
                q_, b_, v_ = state.pop(Ta)
                for tt in range(NCHUNK):
                    cell = {}

                    def a_sc(tt=tt, q=q_, bb=b_, Ta=Ta, cell=cell):
                        cell["pt"] = emit_attn_scores(q, bb, Ta, tt)

                    def a_pv(tt=tt, v=v_, Ta=Ta, cell=cell):
                        emit_attn_pv(cell["pt"], v, Ta, tt)

                    if INTERLEAVE == 2:
                        athunks += [a_sc, a_pv]
                    else:
                        athunks.append(lambda a=a_sc, b=a_pv: (a(), b()))
            if INTERLEAVE and pthunks and athunks:
                for i, th in enumerate(pthunks):
                    th()
                    if INTERLEAVE == 2:
                        if i % 3 != 0 and athunks:
                            athunks.pop(0)()
                    elif i % 3 == 2 and athunks:
                        athunks.pop(0)()
            else:
                for th in pthunks:
                    th()
            for th in athunks:
                th()


def _legalize_sync(nc, max_waits=1):
    """Hoist excess semaphore waits into standalone same-engine
    EventSemaphore instructions. Engine instruction streams execute in
    order, so a wait carried by an immediately-preceding EventSemaphore is
    equivalent to a wait on the instruction itself — and the walrus build
    in this environment rejects instructions with more than one wait."""
    import bass_rust
    n_new = 0
    fn = nc.m.functions[0]
    for blk in fn.blocks:
        out = []
        changed = False
        for ins in blk.instructions:
            si = ins.sync_info
            waits = list(si.on_wait) if si and si.on_wait else []
            if len(waits) > max_waits:
                keep = waits[-max_waits:]
                for w in waits[:-max_waits]:
                    es = mybir.InstEventSemaphore(
                        name=f"esw-{n_new}-{ins.name}", ins=[], outs=[])
                    es.engine = ins.engine
                    es.sync_info = bass_rust.SyncInfo(on_wait=[w], on_update=[])
                    out.append(es)
                    n_new += 1
                ins.sync_info = bass_rust.SyncInfo(
                    on_wait=keep,
                    on_update=list(si.on_update) if si.on_update else [])
                changed = True
            out.append(ins)
        if changed:
            blk.instructions = out
    return n_new


def _build_model(with_bias, iters=1, parts="pas"):
    nc = bass.Bass("TRN2", target_bir_lowering=False, debug=False,
                   enable_partition_id=False)
    xT = nc.dram_tensor("xT", [512, 4096], F16, kind="ExternalInput").ap()
    wq = nc.dram_tensor("wq", [512, 512], F16, kind="ExternalInput").ap()
    wk = nc.dram_tensor("wk", [512, 512], F16, kind="ExternalInput").ap()
    wv = nc.dram_tensor("wv", [512, 512], F16, kind="ExternalInput").ap()
    ebt = nc.dram_tensor("ebt", [128, 64], F16, kind="ExternalInput").ap()
    bqk = (nc.dram_tensor("bqk", [128, 8], F32, kind="ExternalInput").ap()
           if with_bias else None)
    out = nc.dram_tensor("out", [4096, 512], F32, kind="ExternalOutput").ap()
    with tile.TileContext(nc) as tc:
        _emit(tc, out, xT, wq, wk, wv, ebt, bqk, iters=iters, parts=parts)
    return nc


_MODEL_CACHE = {}


def get_model(with_bias=False, legalize=True, iters=1, parts="pas"):
    key = (with_bias, legalize, iters, parts, SCORES_BD, PIPELINE, STORE_SP,
           PROJBUFS, OPSBUFS, EBUFS, INTERLEAVE, SPSBUFS, ATTSPLIT, PMAJOR)
    if key not in _MODEL_CACHE:
        nc = _build_model(with_bias, iters=iters, parts=parts)
        if legalize:
            _legalize_sync(nc)
        _MODEL_CACHE[key] = nc
    return _MODEL_CACHE[key]


def make_in_maps(x, Wq, bq, Wk, bk, Wv, bv, Bbias):
    """Host-side sharding + layout prep. Returns (in_maps, with_bias)."""
    x = np.asarray(x, np.float32)
    with_bias = bool(np.any(bq) or np.any(bk))
    if np.any(bv):
        raise NotImplementedError("nonzero bv not supported")
    wq16 = np.ascontiguousarray(np.asarray(Wq, np.float32).T / 8.0).astype(np.float16)
    wk16 = np.ascontiguousarray(np.asarray(Wk, np.float32).T).astype(np.float16)
    wv16 = np.ascontiguousarray(np.asarray(Wv, np.float32).T).astype(np.float16)
    eb = np.exp(np.asarray(Bbias, np.float32).T)
    ebt = np.concatenate([eb, eb], 0).astype(np.float16)  # [128 (k x2), 64 q]
    common = {"wq": wq16, "wk": wk16, "wv": wv16, "ebt": ebt}
    if with_bias:
        bqk = np.concatenate(
            [np.asarray(bq, np.float32).reshape(4, 128).T / 8.0,
             np.asarray(bk, np.float32).reshape(4, 128).T], 1)  # [128, 8]
        common["bqk"] = np.ascontiguousarray(bqk)
    in_maps = []
    for b in range(B):
        xT16 = np.ascontiguousarray(
            x[b].reshape(TOK, C).T).astype(np.float16)
        in_maps.append({"xT": xT16, **common})
    return in_maps, with_bias


def kernel(**inputs):
    from concourse.bass_utils import run_bass_kernel_spmd
    in_maps, with_bias = make_in_maps(**inputs)
    nc = get_model(with_bias)
    res = run_bass_kernel_spmd(
        nc, in_maps, core_ids=list(range(B)), trace=TRACE)
    LAST["results"] = res
    out = np.stack([r["out"] for r in res.results], 0)
    return out.reshape(B, C, HH, WW)


def _harvest_io(nc):
    import jax
    in_names, out_names, out_avals = [], [], []
    for alloc in nc.m.functions[0].allocations:
        if not isinstance(alloc, mybir.MemoryLocationSet):
            continue
        name = alloc.memorylocations[0].name
        if alloc.kind == "ExternalInput":
            in_names.append(name)
        elif alloc.kind == "ExternalOutput":
            out_names.append(name)
            out_avals.append(jax.core.ShapedArray(
                tuple(alloc.tensor_shape), mybir.dt.np(alloc.dtype)))
    return in_names, out_names, out_avals


def _make_timed_callable(nc, in_maps):
    """Build a jitted shard_map callable around the single bass_exec of
    `nc` (mirrors run_bass_via_pjrt, but with NO donation so the same
    device-resident args can be reused across timed calls; outputs are
    garbage — timing only). Returns a zero-arg closure that runs one
    dispatch and blocks."""
    import jax
    from jax.sharding import Mesh, PartitionSpec
    from jax.experimental.shard_map import shard_map
    from concourse import bass2jax

    bass2jax.install_neuronx_cc_hook()
    in_names, out_names, out_avals = _harvest_io(nc)
    n_params = len(in_names)
    all_names = tuple(in_names + out_names)
    n_cores = len(in_maps)

    def _body(*args):
        return tuple(bass2jax._bass_exec_p.bind(
            *args,
            out_avals=tuple(out_avals),
            in_names=all_names,
            out_names=tuple(out_names),
            lowering_input_output_aliases=(),
            sim_require_finite=True,
            sim_require_nnan=True,
            nc=nc))

    devices = jax.devices()[:n_cores]
    mesh = Mesh(np.asarray(devices), ("core",))
    n_all = n_params + len(out_names)
    sharded = jax.jit(shard_map(
        _body, mesh=mesh,
        in_specs=(PartitionSpec("core"),) * n_all,
        out_specs=(PartitionSpec("core"),) * len(out_names),
        check_rep=False), keep_unused=True)
    concat_in = [
        np.concatenate([np.asarray(m[name]) for m in in_maps], 0)
        for name in in_names]
    concat_zeros = [
        np.zeros((n_cores * a.shape[0], *a.shape[1:]), a.dtype)
        for a in out_avals]
    args = [jax.device_put(a) for a in concat_in + concat_zeros]
    jax.block_until_ready(sharded(*args))  # warm-up / compile

    def run():
        jax.block_until_ready(sharded(*args))
    return run


def time_kernel(inputs, iters=4096, samples=8, parts="pas"):
    """Returns ns per iteration. Builds two model variants — the body run
    once vs `1+iters` times inside an on-device For_i loop — and
    differences median wall-clock over `samples` dispatches of each. With
    ~1s on-device per N-iter dispatch, the ~±20ms axon dispatch jitter
    contributes <2% error."""
    import time
    in_maps, with_bias = make_in_maps(**inputs)
    run1 = _make_timed_callable(
        get_model(with_bias, iters=1, parts=parts), in_maps)
    runN = _make_timed_callable(
        get_model(with_bias, iters=1 + iters, parts=parts), in_maps)
    t1s, tNs = [], []
    for _ in range(samples):
        t0 = time.time(); run1(); t1s.append(time.time() - t0)
        t0 = time.time(); runN(); tNs.append(time.time() - t0)
    t1 = float(np.median(t1s)); tN = float(np.median(tNs))
    return (tN - t1) / iters * 1e9, (t1s, tNs)



# revision 77
# speedup vs baseline: 4.0523x; 1.0010x over previous
"""Windowed multi-head attention TRN2 kernel (Bass/Tile), SPMD over 8 cores.

Problem (per reference): x:(8,512,64,64) viewed as (B, 4096 tok, 512 c);
Q/K/V = tok @ W^T + b; per window (64 tok) & head (8 x 64d):
softmax(QK^T/8 + Bbias) @ V; output back in (B,512,64,64).

Sharding: data-parallel, one batch element per core (8 cores).

Per-core dataflow (all matmuls fp16 operands, fp32 PSUM accum):
 - host passes x^T (c, tok) fp16 so projection rhs tiles DMA contiguously
 - Q^T,K^T computed in [c_out, tok] layout (heads pairs on partition halves)
 - V computed in natural [tok, c] layout, with a per-head ones-column
   appended (65-wide head blocks) so PV matmuls also produce softmax
   denominators
 - scores^T = K^T_wh^T-matmul: [k,q] tiles packed 8 units/PSUM bank,
   head-parity (e) on partition halves
 - softmax without max-subtraction (scores are O(1)): exp on ACT; the
   exp(Bbias^T) elementwise multiply on DVE simultaneously moves probs
   to window-parity (p) partition halves, so PV runs against natural V
   (no duplicate) and outputs land in natural token rows
 - PV: [64q, 65] units, 4 per PSUM bank; normalize via one batched
   reciprocal + one batched multiply per bank during PSUM->SBUF evac
"""

import sys
import numpy as np

for _p in ("/opt/trn_rl_repo",):
    if _p not in sys.path:
        sys.path.insert(0, _p)

from contextlib import ExitStack

import concourse.bass as bass
import concourse.tile as tile
from concourse import mybir

F16 = mybir.dt.float16
F32 = mybir.dt.float32

B, C, HH, WW = 8, 512, 64, 64
NH, HD = 8, 64
WIN = 64            # tokens per window
TOK = C * 0 + 4096  # tokens per batch/core
NT = 8              # 512-token tiles per core
NCHUNK = 4          # 128-channel chunks

TRACE = False
LAST = {}
SCORES_BD = True  # scores via block-diagonal K (full 128-part contraction)
PIPELINE = 1      # emit attention this many T-tiles behind projections
STORE_SP = True   # issue output stores from SP instead of ACT
PROJBUFS = 3      # PSUM banks for projection groups
OPSBUFS = 1       # PSUM buffers per PV output tag (2 tags)
SPSBUFS = 1       # PSUM buffers per scores tag
ATTSPLIT = 1      # split exp/prob-mul for finer attention wavefront
PMAJOR = 1        # scores matmuls in window-parity-major order
ONBUFS = 2        # SBUF buffers for output staging tiles (2 tags)
SSPLIT = 1        # separate scores PSUM tile per window parity
EBUFS = 2         # SBUF buffers for attention et/pt/on tiles
INTERLEAVE = 1    # emit attention subtiles between projection groups


def _emit(tc, out, xT, wq, wk, wv, ebt, bqk, iters=1, parts="pas"):
    """Emit the per-core program. bqk: [128, 8] fp32 (bq/8 | bk chunks) or None.
    parts: subset of 'p' (projections), 'a' (attention), 's' (store) for
    timing ablations."""
    nc = tc.nc
    Exp = mybir.ActivationFunctionType.Exp
    Ident = mybir.ActivationFunctionType.Identity

    with ExitStack() as ctx:
        ep = ctx.enter_context

        tbufs = PIPELINE + 1
        wpool = ep(tc.tile_pool(name="w", bufs=1))
        xpool = ep(tc.tile_pool(name="x", bufs=tbufs))
        qkpool = ep(tc.tile_pool(name="qk", bufs=tbufs))
        vpool = ep(tc.tile_pool(name="v", bufs=tbufs))
        epool = ep(tc.tile_pool(name="e", bufs=EBUFS))
        bdpool = ep(tc.tile_pool(name="bd", bufs=tbufs))
        rcpool = ep(tc.tile_pool(name="rc", bufs=4))
        onpool = ep(tc.tile_pool(name="on", bufs=ONBUFS))
        projps = ep(tc.tile_pool(name="projps", bufs=PROJBUFS, space="PSUM"))
        sps = ep(tc.tile_pool(name="sps", bufs=SPSBUFS, space="PSUM"))
        ops = ep(tc.tile_pool(name="ops", bufs=OPSBUFS, space="PSUM"))

        # resident weights: [c_in chunk 128, c_out 512] fp16 per proj
        wsb = {}
        for nm, wdram in (("q", wq), ("k", wk), ("v", wv)):
            for ci in range(NCHUNK):
                t = wpool.tile([128, 512], F16, tag=f"w{nm}{ci}")
                nc.sync.dma_start(t[:], wdram[ci * 128:(ci + 1) * 128, :])
                wsb[nm, ci] = t
        ebt_sb = wpool.tile([128, 64], F16, tag="ebt")
        nc.sync.dma_start(ebt_sb[:], ebt[:, :])
        bqk_sb = None
        if bqk is not None:
            bqk_sb = wpool.tile([128, 8], F32, tag="bqk")
            nc.sync.dma_start(bqk_sb[:], bqk[:, :])

        # ---- one-time inits: zero blocks of block-diagonal tiles and the
        # ones-columns of V survive every iteration (later writes only
        # touch the data blocks), so initialize all rotating buffers here,
        # outside the timing loop. Tile calls advance each tag's rotation
        # by bufs=2, preserving in-loop phase.
        for i in range(max(tbufs, EBUFS)):
            if SCORES_BD and i < tbufs:
                for j in range(4):
                    t = bdpool.tile([128, 1024], F16, tag=f"bdk{j}")
                    nc.gpsimd.memset(t[:], 0)
            if i < EBUFS:
                t = epool.tile([128, 1024], F16, tag="pt")
                nc.gpsimd.memset(t[:], 0)
            if i < tbufs:
                for tt in range(NCHUNK):
                    vn = vpool.tile([128, 520], F16, tag=f"vn{tt}")
                    nc.scalar.activation(
                        vn[:].rearrange("p (h x) -> p h x", x=65)[:, :, 64],
                        ebt_sb[:, 0:8], Ident, bias=1.0, scale=0.0)

        if iters > 1:
            # on-device repetition for timing: amortizes host dispatch
            ep(tc.For_i(0, iters))

        def emit_proj_group(nm, co, xt, dst, bdk=None):
            """One projection PSUM group (4 matmuls) + its evacuation."""
            pi = 0 if nm == "q" else 1
            ps = projps.tile([128, 512], F32, tag="proj")
            if nm == "v":
                for ci in range(NCHUNK):
                    nc.tensor.matmul(
                        ps[:],
                        xt[ci][:, co * 128:(co + 1) * 128],
                        wsb["v", ci][:],
                        start=(ci == 0), stop=(ci == NCHUNK - 1))
                if "P" not in parts:
                    nc.vector.tensor_copy(
                        dst[:].rearrange("p (h x) -> p h x", x=65)
                        [:, :, 0:64],
                        ps[:].rearrange("p (h x) -> p h x", x=64))
                return
            for ci in range(NCHUNK):
                w_ap = (wsb[nm, 0][:, 0:128] if "W" in parts
                        else wsb[nm, ci][:, co * 128:(co + 1) * 128])
                if "A" in parts:  # ablation: independent matmuls, no accum
                    nc.tensor.matmul(
                        ps[:], w_ap, xt[0 if "W" in parts else ci][:],
                        start=True, stop=True)
                else:
                    nc.tensor.matmul(
                        ps[:], w_ap, xt[0 if "W" in parts else ci][:],
                        start=(ci == 0), stop=(ci == NCHUNK - 1))
            if "P" in parts:
                return
            if bdk is not None:
                # K straight into block-diagonal layout, window-major:
                # col g*128 + e*64 + k, g = 2tt+p
                bd_v = bdk[:].rearrange("r (g c) -> r g c", c=128)
                ps_v = ps[:].rearrange("r (g k) -> r g k", k=64)
                for e in range(2):
                    re = slice(e * 64, e * 64 + 64)
                    if bqk_sb is not None:
                        nc.scalar.activation(
                            bd_v[re, :, e * 64:e * 64 + 64],
                            ps_v[re], Ident,
                            bias=bqk_sb[re, 4 + co:5 + co])
                    else:
                        nc.scalar.copy(
                            bd_v[re, :, e * 64:e * 64 + 64], ps_v[re])
            elif bqk_sb is not None:
                nc.scalar.activation(
                    dst[:], ps[:], Ident,
                    bias=bqk_sb[:, pi * 4 + co:pi * 4 + co + 1])
            else:
                nc.scalar.copy(dst[:], ps[:])

        def proj_thunks(T):
            """xt loads (immediate) + 12 emission thunks for T's
            projection PSUM groups; returns (thunks, state_entry)."""
            xt = []
            for ci in range(NCHUNK):
                t = xpool.tile([128, 512], F16, tag=f"xt{ci}")
                nc.sync.dma_start(
                    t[:],
                    xT[ci * 128:(ci + 1) * 128, T * 512:(T + 1) * 512])
                xt.append(t)
            qkt = {}
            bdks = []
            vnat = []
            thunks = []

            def emit_proj_pair(nm, co0, xt):
                # ablation: interleave two groups' accumulation chains so
                # consecutive matmuls never hit the same PSUM region
                ps0 = projps.tile([128, 512], F32, tag="proj")
                ps1 = projps.tile([128, 512], F32, tag="proj")
                for ci in range(NCHUNK):
                    for k, ps in ((0, ps0), (1, ps1)):
                        co = co0 + k
                        nc.tensor.matmul(
                            ps[:],
                            wsb[nm, ci][:, co * 128:(co + 1) * 128],
                            xt[ci][:],
                            start=(ci == 0), stop=(ci == NCHUNK - 1))

            names = ("q",) if SCORES_BD else ("q", "k")
            for nm in names:
                if "I" in parts and "p" in parts:
                    for co0 in (0, 2):
                        thunks.append(
                            lambda nm=nm, co0=co0: emit_proj_pair(
                                nm, co0, xt))
                    for co in range(NCHUNK):
                        t = qkpool.tile([128, 512], F16, tag=f"{nm}t{co}")
                        qkt[nm, co] = t
                    continue
                for co in range(NCHUNK):
                    t = qkpool.tile([128, 512], F16, tag=f"{nm}t{co}")
                    qkt[nm, co] = t
                    if "p" in parts:
                        thunks.append(
                            lambda nm=nm, co=co, t=t: emit_proj_group(
                                nm, co, xt, t))
            if SCORES_BD:
                for j in range(4):
                    bdk = bdpool.tile([128, 1024], F16, tag=f"bdk{j}")
                    bdks.append(bdk)
                    if "p" in parts:
                        thunks.append(
                            lambda j=j, bdk=bdk: emit_proj_group(
                                "k", j, xt, None, bdk=bdk))
            for tt in range(NCHUNK):
                vn = vpool.tile([128, 520], F16, tag=f"vn{tt}")
                vnat.append(vn)
                if "p" in parts:
                    thunks.append(
                        lambda tt=tt, vn=vn: emit_proj_group(
                            "v", tt, xt, vn))
            return thunks, (qkt, bdks, vnat)

        def emit_attn_scores(qkt, bdks, Ta, tt):
            # ---- attention: subtile tt covers windows 2tt, 2tt+1 of Ta.
            # HAZARD RULE: concurrent matmuls with disjoint row-groups but
            # a shared column-group collide in the PE array (device crash);
            # sub-128 matmuls are placed DIAGONALLY (out partition base ==
            # operand partition base). Scores land head-parity packed (e on
            # halves); the exp(Bbias)-multiply on DVE moves probs to
            # block-diagonal window-parity layout, so PV runs full-width
            # against natural V and outputs land in natural token rows.
            if True:
                if SSPLIT and SCORES_BD:
                    # separate PSUM tile per window parity so exp of parity
                    # p depends on only its own 4 matmuls even with
                    # tile-granular PSUM dependency tracking
                    s2 = []
                    for p in range(2):
                        s_half = sps.tile([128, 256], F32, tag=f"s{p}")
                        s2.append(s_half)
                    for p in range(2):
                        for j in range(4):
                            w = 2 * tt + p
                            nc.tensor.matmul(
                                s2[p][:, j * 64:(j + 1) * 64],
                                bdks[j][:, tt * 256 + p * 128:
                                        tt * 256 + (p + 1) * 128],
                                qkt["q", j][:, w * 64:(w + 1) * 64],
                                start=True, stop=True)
                    s = None
                else:
                    s = sps.tile([128, 512], F32, tag="s")
                if SCORES_BD and not SSPLIT:
                    # p-major: with ATTSPLIT, exp of parity p waits only on
                    # its own 4 matmuls
                    for p, j in (
                            [(p, j) for p in range(2) for j in range(4)]
                            if PMAJOR else
                            [(p, j) for j in range(4) for p in range(2)]):
                        if True:
                            w = 2 * tt + p
                            nc.tensor.matmul(
                                s[:, (j * 2 + p) * 64:(j * 2 + p + 1) * 64],
                                bdks[j][:, tt * 256 + p * 128:
                                        tt * 256 + (p + 1) * 128],
                                qkt["q", j][:, w * 64:(w + 1) * 64],
                                start=True, stop=True)
                elif not SCORES_BD:
                    for j in range(4):
                        for e in range(2):
                            r = slice(e * 64, e * 64 + 64)
                            for p in range(2):
                                w = 2 * tt + p
                                wc = slice(w * 64, w * 64 + 64)
                                nc.tensor.matmul(
                                    s[r, (j * 2 + p) * 64:
                                      (j * 2 + p + 1) * 64],
                                    qkt["k", j][r, wc],
                                    qkt["q", j][r, wc],
                                    start=True, stop=True)
                et = epool.tile([128, 512], F16, tag="et")
                et_v = et[:].rearrange("r (j u q) -> r j u q", u=2, q=64)
                if SSPLIT and SCORES_BD:
                    for p in range(2):
                        nc.scalar.activation(
                            et_v[:, :, p, :],
                            s2[p][:].rearrange("r (j q) -> r j q", q=64),
                            Exp)
                elif ATTSPLIT:
                    # split exp per window parity: the (p, e) multiplies
                    # depend only on their own exp half
                    s_v = s[:].rearrange("r (j u q) -> r j u q", u=2, q=64)
                    for p in range(2):
                        nc.scalar.activation(
                            et_v[:, :, p, :], s_v[:, :, p, :], Exp)
                else:
                    nc.scalar.activation(et[:], s[:], Exp)
                # block-diagonal probs: pt[p*64+k, h*128+p*64+q] =
                # et[e*64+k, (2j+p)*64+q]*ebt[k,q] (h=2j+e); off-diagonal
                # blocks stay zero, so one PV matmul covers both windows
                # with full 128-partition contraction against natural V.
                pt = epool.tile([128, 1024], F16, tag="pt")
                pt_v = pt[:].rearrange("r (j z) -> r j z", j=4)
                for p in range(2):
                    rp = slice(p * 64, p * 64 + 64)
                    for e in range(2):
                        re = slice(e * 64, e * 64 + 64)
                        c0 = e * 128 + p * 64
                        if ATTSPLIT:
                            # split per bank-half: PV bank b waits only on
                            # the j in {2b, 2b+1} multiplies
                            for bh in range(2):
                                js = slice(bh * 2, bh * 2 + 2)
                                nc.vector.tensor_mul(
                                    pt_v[rp, js, c0:c0 + 64],
                                    et_v[re, js, p, :],
                                    ebt_sb[re, 0:64].unsqueeze(1)
                                    .broadcast_to((64, 2, 64)))
                        else:
                            nc.vector.tensor_mul(
                                pt_v[rp, :, c0:c0 + 64],
                                et_v[re, :, p, :],
                                ebt_sb[re, 0:64].unsqueeze(1)
                                .broadcast_to((64, 4, 64)))
                return pt

        def emit_attn_pv(pt, vnat, Ta, tt):
            if True:
                # PV: 8 matmuls (one per head), full 128 partitions; two
                # PSUM banks of 4 [128q2w, 65] units each.
                on = onpool.tile([128, 512], F32, tag=f"on{tt % 2}")
                for b in range(2):
                    o = ops.tile([128, 260], F32, tag=f"ob{b}")
                    o_v = o[:].rearrange("r (u x) -> r u x", x=65)
                    for u in range(4):
                        h = 4 * b + u
                        nc.tensor.matmul(
                            o[:, u * 65:(u + 1) * 65],
                            pt[:, h * 128:(h + 1) * 128],
                            vnat[tt][:, h * 65:(h + 1) * 65],
                            start=True, stop=True)
                    rc = rcpool.tile([128, 4], F32, tag=f"rc{b}")
                    nc.vector.reciprocal(rc[:, 0:4], o_v[:, :, 64])
                    nc.vector.tensor_mul(
                        on[:].rearrange("r (b2 u q) -> r b2 u q", b2=2, q=64)
                        [:, b, :, :],
                        o_v[:, :, 0:64],
                        rc[:, 0:4].unsqueeze(2).broadcast_to((128, 4, 64)))
                if "s" in parts:
                    eng = nc.sync if STORE_SP else nc.scalar
                    eng.dma_start(
                        out[Ta * 512 + tt * 128: Ta * 512 + (tt + 1) * 128,
                            :],
                        on[:])

        # software pipeline driver: attention trails projections by
        # PIPELINE T-tiles; with INTERLEAVE, attention subtiles are emitted
        # between projection groups as scheduler priority hints.
        state = {}
        for T in range(NT + PIPELINE):
            pthunks = []
            if T < NT:
                pthunks, entry = proj_thunks(T)
                state[T] = entry
            athunks = []
            if T >= PIPELINE and "a" in parts:
                Ta = T - PIPELINE
                q_, b_, v_ = state.pop(Ta)
                for tt in range(NCHUNK):
                    cell = {}

                    def a_sc(tt=tt, q=q_, bb=b_, Ta=Ta, cell=cell):
                        cell["pt"] = emit_attn_scores(q, bb, Ta, tt)

                    def a_pv(tt=tt, v=v_, Ta=Ta, cell=cell):
                        emit_attn_pv(cell["pt"], v, Ta, tt)

                    if INTERLEAVE == 2:
                        athunks += [a_sc, a_pv]
                    else:
                        athunks.append(lambda a=a_sc, b=a_pv: (a(), b()))
            if INTERLEAVE and pthunks and athunks:
                for i, th in enumerate(pthunks):
                    th()
                    if INTERLEAVE == 2:
                        if i % 3 != 0 and athunks:
                            athunks.pop(0)()
                    elif i % 3 == 2 and athunks:
                        athunks.pop(0)()
            else:
                for th in pthunks:
                    th()
            for th in athunks:
                th()


def _legalize_sync(nc, max_waits=1):
    """Hoist excess semaphore waits into standalone same-engine
    EventSemaphore instructions. Engine instruction streams execute in
    order, so a wait carried by an immediately-preceding EventSemaphore is
    equivalent to a wait on the instruction itself — and the walrus build
    in this environment rejects instructions with more than one wait."""
    import bass_rust
    n_new = 0
    fn = nc.m.functions[0]
    for blk in fn.blocks:
        out = []
        changed = False
        for ins in blk.instructions:
            si = ins.sync_info
            waits = list(si.on_wait) if si and si.on_wait else []
            if len(waits) > max_waits:
                keep = waits[-max_waits:]
                for w in waits[:-max_waits]:
                    es = mybir.InstEventSemaphore(
                        name=f"esw-{n_new}-{ins.name}", ins=[], outs=[])
                    es.engine = ins.engine
                    es.sync_info = bass_rust.SyncInfo(on_wait=[w], on_update=[])
                    out.append(es)
                    n_new += 1
                ins.sync_info = bass_rust.SyncInfo(
                    on_wait=keep,
                    on_update=list(si.on_update) if si.on_update else [])
                changed = True
            out.append(ins)
        if changed:
            blk.instructions = out
    return n_new


def _build_model(with_bias, iters=1, parts="pas"):
    nc = bass.Bass("TRN2", target_bir_lowering=False, debug=False,
                   enable_partition_id=False)
    xT = nc.dram_tensor("xT", [512, 4096], F16, kind="ExternalInput").ap()
    wq = nc.dram_tensor("wq", [512, 512], F16, kind="ExternalInput").ap()
    wk = nc.dram_tensor("wk", [512, 512], F16, kind="ExternalInput").ap()
    wv = nc.dram_tensor("wv", [512, 512], F16, kind="ExternalInput").ap()
    ebt = nc.dram_tensor("ebt", [128, 64], F16, kind="ExternalInput").ap()
    bqk = (nc.dram_tensor("bqk", [128, 8], F32, kind="ExternalInput").ap()
           if with_bias else None)
    out = nc.dram_tensor("out", [4096, 512], F32, kind="ExternalOutput").ap()
    with tile.TileContext(nc) as tc:
        _emit(tc, out, xT, wq, wk, wv, ebt, bqk, iters=iters, parts=parts)
    return nc


_MODEL_CACHE = {}


def get_model(with_bias=False, legalize=True, iters=1, parts="pas"):
    key = (with_bias, legalize, iters, parts, SCORES_BD, PIPELINE, STORE_SP,
           PROJBUFS, OPSBUFS, EBUFS, INTERLEAVE, SPSBUFS, ATTSPLIT, PMAJOR,
           ONBUFS, SSPLIT)
    if key not in _MODEL_CACHE:
        nc = _build_model(with_bias, iters=iters, parts=parts)
        if legalize:
            _legalize_sync(nc)
        _MODEL_CACHE[key] = nc
    return _MODEL_CACHE[key]


def make_in_maps(x, Wq, bq, Wk, bk, Wv, bv, Bbias):
    """Host-side sharding + layout prep. Returns (in_maps, with_bias)."""
    x = np.asarray(x, np.float32)
    with_bias = bool(np.any(bq) or np.any(bk))
    if np.any(bv):
        raise NotImplementedError("nonzero bv not supported")
    wq16 = np.ascontiguousarray(np.asarray(Wq, np.float32).T / 8.0).astype(np.float16)
    wk16 = np.ascontiguousarray(np.asarray(Wk, np.float32).T).astype(np.float16)
    wv16 = np.ascontiguousarray(np.asarray(Wv, np.float32).T).astype(np.float16)
    eb = np.exp(np.asarray(Bbias, np.float32).T)
    ebt = np.concatenate([eb, eb], 0).astype(np.float16)  # [128 (k x2), 64 q]
    common = {"wq": wq16, "wk": wk16, "wv": wv16, "ebt": ebt}
    if with_bias:
        bqk = np.concatenate(
            [np.asarray(bq, np.float32).reshape(4, 128).T / 8.0,
             np.asarray(bk, np.float32).reshape(4, 128).T], 1)  # [128, 8]
        common["bqk"] = np.ascontiguousarray(bqk)
    in_maps = []
    for b in range(B):
        xT16 = np.ascontiguousarray(
            x[b].reshape(TOK, C).T).astype(np.float16)
        in_maps.append({"xT": xT16, **common})
    return in_maps, with_bias


def kernel(**inputs):
    from concourse.bass_utils import run_bass_kernel_spmd
    in_maps, with_bias = make_in_maps(**inputs)
    nc = get_model(with_bias)
    res = run_bass_kernel_spmd(
        nc, in_maps, core_ids=list(range(B)), trace=TRACE)
    LAST["results"] = res
    out = np.stack([r["out"] for r in res.results], 0)
    return out.reshape(B, C, HH, WW)


def _harvest_io(nc):
    import jax
    in_names, out_names, out_avals = [], [], []
    for alloc in nc.m.functions[0].allocations:
        if not isinstance(alloc, mybir.MemoryLocationSet):
            continue
        name = alloc.memorylocations[0].name
        if alloc.kind == "ExternalInput":
            in_names.append(name)
        elif alloc.kind == "ExternalOutput":
            out_names.append(name)
            out_avals.append(jax.core.ShapedArray(
                tuple(alloc.tensor_shape), mybir.dt.np(alloc.dtype)))
    return in_names, out_names, out_avals


def _make_timed_callable(nc, in_maps):
    """Build a jitted shard_map callable around the single bass_exec of
    `nc` (mirrors run_bass_via_pjrt, but with NO donation so the same
    device-resident args can be reused across timed calls; outputs are
    garbage — timing only). Returns a zero-arg closure that runs one
    dispatch and blocks."""
    import jax
    from jax.sharding import Mesh, PartitionSpec
    from jax.experimental.shard_map import shard_map
    from concourse import bass2jax

    bass2jax.install_neuronx_cc_hook()
    in_names, out_names, out_avals = _harvest_io(nc)
    n_params = len(in_names)
    all_names = tuple(in_names + out_names)
    n_cores = len(in_maps)

    def _body(*args):
        return tuple(bass2jax._bass_exec_p.bind(
            *args,
            out_avals=tuple(out_avals),
            in_names=all_names,
            out_names=tuple(out_names),
            lowering_input_output_aliases=(),
            sim_require_finite=True,
            sim_require_nnan=True,
            nc=nc))

    devices = jax.devices()[:n_cores]
    mesh = Mesh(np.asarray(devices), ("core",))
    n_all = n_params + len(out_names)
    sharded = jax.jit(shard_map(
        _body, mesh=mesh,
        in_specs=(PartitionSpec("core"),) * n_all,
        out_specs=(PartitionSpec("core"),) * len(out_names),
        check_rep=False), keep_unused=True)
    concat_in = [
        np.concatenate([np.asarray(m[name]) for m in in_maps], 0)
        for name in in_names]
    concat_zeros = [
        np.zeros((n_cores * a.shape[0], *a.shape[1:]), a.dtype)
        for a in out_avals]
    args = [jax.device_put(a) for a in concat_in + concat_zeros]
    jax.block_until_ready(sharded(*args))  # warm-up / compile

    def run():
        jax.block_until_ready(sharded(*args))
    return run


def time_kernel(inputs, iters=4096, samples=8, parts="pas"):
    """Returns ns per iteration. Builds two model variants — the body run
    once vs `1+iters` times inside an on-device For_i loop — and
    differences median wall-clock over `samples` dispatches of each. With
    ~1s on-device per N-iter dispatch, the ~±20ms axon dispatch jitter
    contributes <2% error."""
    import time
    in_maps, with_bias = make_in_maps(**inputs)
    run1 = _make_timed_callable(
        get_model(with_bias, iters=1, parts=parts), in_maps)
    runN = _make_timed_callable(
        get_model(with_bias, iters=1 + iters, parts=parts), in_maps)
    t1s, tNs = [], []
    for _ in range(samples):
        t0 = time.time(); run1(); t1s.append(time.time() - t0)
        t0 = time.time(); runN(); tNs.append(time.time() - t0)
    t1 = float(np.median(t1s)); tN = float(np.median(tNs))
    return (tN - t1) / iters * 1e9, (t1s, tNs)



# revision 82
# speedup vs baseline: 4.1289x; 1.0189x over previous
"""Windowed multi-head attention TRN2 kernel (Bass/Tile), SPMD over 8 cores.

Problem (per reference): x:(8,512,64,64) viewed as (B, 4096 tok, 512 c);
Q/K/V = tok @ W^T + b; per window (64 tok) & head (8 x 64d):
softmax(QK^T/8 + Bbias) @ V; output back in (B,512,64,64).

Sharding: data-parallel, one batch element per core (8 cores).

Per-core dataflow (all matmuls fp16 operands, fp32 PSUM accum):
 - host passes x^T (c, tok) fp16 so projection rhs tiles DMA contiguously
 - Q^T,K^T computed in [c_out, tok] layout (heads pairs on partition halves)
 - V computed in natural [tok, c] layout, with a per-head ones-column
   appended (65-wide head blocks) so PV matmuls also produce softmax
   denominators
 - scores^T = K^T_wh^T-matmul: [k,q] tiles packed 8 units/PSUM bank,
   head-parity (e) on partition halves
 - softmax without max-subtraction (scores are O(1)): exp on ACT; the
   exp(Bbias^T) elementwise multiply on DVE simultaneously moves probs
   to window-parity (p) partition halves, so PV runs against natural V
   (no duplicate) and outputs land in natural token rows
 - PV: [64q, 65] units, 4 per PSUM bank; normalize via one batched
   reciprocal + one batched multiply per bank during PSUM->SBUF evac
"""

import sys
import numpy as np

for _p in ("/opt/trn_rl_repo",):
    if _p not in sys.path:
        sys.path.insert(0, _p)

from contextlib import ExitStack

import concourse.bass as bass
import concourse.tile as tile
from concourse import mybir

F16 = mybir.dt.float16
F32 = mybir.dt.float32

B, C, HH, WW = 8, 512, 64, 64
NH, HD = 8, 64
WIN = 64            # tokens per window
TOK = C * 0 + 4096  # tokens per batch/core
NT = 8              # 512-token tiles per core
NCHUNK = 4          # 128-channel chunks

TRACE = False
LAST = {}
SCORES_BD = True  # scores via block-diagonal K (full 128-part contraction)
PIPELINE = 1      # emit attention this many T-tiles behind projections
STORE_SP = True   # issue output stores from SP instead of ACT
PROJBUFS = 3      # PSUM banks for projection groups
OPSBUFS = 1       # PSUM buffers per PV output tag (2 tags)
SPSBUFS = 1       # PSUM buffers per scores tag
ATTSPLIT = 1      # split exp/prob-mul for finer attention wavefront
PMAJOR = 1        # scores matmuls in window-parity-major order
ONBUFS = 2        # SBUF buffers for output staging tiles (2 tags)
SSPLIT = 1        # separate scores PSUM tile per window parity
S0DOUBLE = 1      # double-buffer parity-0 scores (uses the spare bank)
EBUFS = 2         # SBUF buffers for attention et/pt/on tiles
INTERLEAVE = 1    # emit attention subtiles between projection groups


def _emit(tc, out, xT, wq, wk, wv, ebt, bqk, iters=1, parts="pas"):
    """Emit the per-core program. bqk: [128, 8] fp32 (bq/8 | bk chunks) or None.
    parts: subset of 'p' (projections), 'a' (attention), 's' (store) for
    timing ablations."""
    nc = tc.nc
    Exp = mybir.ActivationFunctionType.Exp
    Ident = mybir.ActivationFunctionType.Identity

    with ExitStack() as ctx:
        ep = ctx.enter_context

        tbufs = PIPELINE + 1
        wpool = ep(tc.tile_pool(name="w", bufs=1))
        xpool = ep(tc.tile_pool(name="x", bufs=tbufs))
        qkpool = ep(tc.tile_pool(name="qk", bufs=tbufs))
        vpool = ep(tc.tile_pool(name="v", bufs=tbufs))
        epool = ep(tc.tile_pool(name="e", bufs=EBUFS))
        bdpool = ep(tc.tile_pool(name="bd", bufs=tbufs))
        rcpool = ep(tc.tile_pool(name="rc", bufs=4))
        onpool = ep(tc.tile_pool(name="on", bufs=ONBUFS))
        projps = ep(tc.tile_pool(name="projps", bufs=PROJBUFS, space="PSUM"))
        sps = ep(tc.tile_pool(name="sps", bufs=SPSBUFS, space="PSUM"))
        # one PSUM bank is spare: optionally double-buffer parity-0 scores
        sps0 = (ep(tc.tile_pool(name="sps0", bufs=2, space="PSUM"))
                if S0DOUBLE and SSPLIT else sps)
        ops = ep(tc.tile_pool(name="ops", bufs=OPSBUFS, space="PSUM"))

        # resident weights: [c_in chunk 128, c_out 512] fp16 per proj
        wsb = {}
        for nm, wdram in (("q", wq), ("k", wk), ("v", wv)):
            for ci in range(NCHUNK):
                t = wpool.tile([128, 512], F16, tag=f"w{nm}{ci}")
                nc.sync.dma_start(t[:], wdram[ci * 128:(ci + 1) * 128, :])
                wsb[nm, ci] = t
        ebt_sb = wpool.tile([128, 64], F16, tag="ebt")
        nc.sync.dma_start(ebt_sb[:], ebt[:, :])
        bqk_sb = None
        if bqk is not None:
            bqk_sb = wpool.tile([128, 8], F32, tag="bqk")
            nc.sync.dma_start(bqk_sb[:], bqk[:, :])

        # ---- one-time inits: zero blocks of block-diagonal tiles and the
        # ones-columns of V survive every iteration (later writes only
        # touch the data blocks), so initialize all rotating buffers here,
        # outside the timing loop. Tile calls advance each tag's rotation
        # by bufs=2, preserving in-loop phase.
        for i in range(max(tbufs, EBUFS)):
            if SCORES_BD and i < tbufs:
                for j in range(4):
                    t = bdpool.tile([128, 1024], F16, tag=f"bdk{j}")
                    nc.gpsimd.memset(t[:], 0)
            if i < EBUFS:
                t = epool.tile([128, 1024], F16, tag="pt")
                nc.gpsimd.memset(t[:], 0)
            if i < tbufs:
                for tt in range(NCHUNK):
                    vn = vpool.tile([128, 520], F16, tag=f"vn{tt}")
                    nc.scalar.activation(
                        vn[:].rearrange("p (h x) -> p h x", x=65)[:, :, 64],
                        ebt_sb[:, 0:8], Ident, bias=1.0, scale=0.0)

        if iters > 1:
            # on-device repetition for timing: amortizes host dispatch
            ep(tc.For_i(0, iters))

        def emit_proj_group(nm, co, xt, dst, bdk=None):
            """One projection PSUM group (4 matmuls) + its evacuation."""
            pi = 0 if nm == "q" else 1
            ps = projps.tile([128, 512], F32, tag="proj")
            if nm == "v":
                for ci in range(NCHUNK):
                    nc.tensor.matmul(
                        ps[:],
                        xt[ci][:, co * 128:(co + 1) * 128],
                        wsb["v", ci][:],
                        start=(ci == 0), stop=(ci == NCHUNK - 1))
                if "P" not in parts:
                    nc.vector.tensor_copy(
                        dst[:].rearrange("p (h x) -> p h x", x=65)
                        [:, :, 0:64],
                        ps[:].rearrange("p (h x) -> p h x", x=64))
                return
            for ci in range(NCHUNK):
                w_ap = (wsb[nm, 0][:, 0:128] if "W" in parts
                        else wsb[nm, ci][:, co * 128:(co + 1) * 128])
                if "A" in parts:  # ablation: independent matmuls, no accum
                    nc.tensor.matmul(
                        ps[:], w_ap, xt[0 if "W" in parts else ci][:],
                        start=True, stop=True)
                else:
                    nc.tensor.matmul(
                        ps[:], w_ap, xt[0 if "W" in parts else ci][:],
                        start=(ci == 0), stop=(ci == NCHUNK - 1))
            if "P" in parts:
                return
            if bdk is not None:
                # K straight into block-diagonal layout, window-major:
                # col g*128 + e*64 + k, g = 2tt+p
                bd_v = bdk[:].rearrange("r (g c) -> r g c", c=128)
                ps_v = ps[:].rearrange("r (g k) -> r g k", k=64)
                for e in range(2):
                    re = slice(e * 64, e * 64 + 64)
                    if bqk_sb is not None:
                        nc.scalar.activation(
                            bd_v[re, :, e * 64:e * 64 + 64],
                            ps_v[re], Ident,
                            bias=bqk_sb[re, 4 + co:5 + co])
                    else:
                        nc.scalar.copy(
                            bd_v[re, :, e * 64:e * 64 + 64], ps_v[re])
            elif bqk_sb is not None:
                nc.scalar.activation(
                    dst[:], ps[:], Ident,
                    bias=bqk_sb[:, pi * 4 + co:pi * 4 + co + 1])
            else:
                nc.scalar.copy(dst[:], ps[:])

        def proj_thunks(T):
            """xt loads (immediate) + 12 emission thunks for T's
            projection PSUM groups; returns (thunks, state_entry)."""
            xt = []
            for ci in range(NCHUNK):
                t = xpool.tile([128, 512], F16, tag=f"xt{ci}")
                nc.sync.dma_start(
                    t[:],
                    xT[ci * 128:(ci + 1) * 128, T * 512:(T + 1) * 512])
                xt.append(t)
            qkt = {}
            bdks = []
            vnat = []
            thunks = []

            def emit_proj_pair(nm, co0, xt):
                # ablation: interleave two groups' accumulation chains so
                # consecutive matmuls never hit the same PSUM region
                ps0 = projps.tile([128, 512], F32, tag="proj")
                ps1 = projps.tile([128, 512], F32, tag="proj")
                for ci in range(NCHUNK):
                    for k, ps in ((0, ps0), (1, ps1)):
                        co = co0 + k
                        nc.tensor.matmul(
                            ps[:],
                            wsb[nm, ci][:, co * 128:(co + 1) * 128],
                            xt[ci][:],
                            start=(ci == 0), stop=(ci == NCHUNK - 1))

            names = ("q",) if SCORES_BD else ("q", "k")
            for nm in names:
                if "I" in parts and "p" in parts:
                    for co0 in (0, 2):
                        thunks.append(
                            lambda nm=nm, co0=co0: emit_proj_pair(
                                nm, co0, xt))
                    for co in range(NCHUNK):
                        t = qkpool.tile([128, 512], F16, tag=f"{nm}t{co}")
                        qkt[nm, co] = t
                    continue
                for co in range(NCHUNK):
                    t = qkpool.tile([128, 512], F16, tag=f"{nm}t{co}")
                    qkt[nm, co] = t
                    if "p" in parts:
                        thunks.append(
                            lambda nm=nm, co=co, t=t: emit_proj_group(
                                nm, co, xt, t))
            if SCORES_BD:
                for j in range(4):
                    bdk = bdpool.tile([128, 1024], F16, tag=f"bdk{j}")
                    bdks.append(bdk)
                    if "p" in parts:
                        thunks.append(
                            lambda j=j, bdk=bdk: emit_proj_group(
                                "k", j, xt, None, bdk=bdk))
            for tt in range(NCHUNK):
                vn = vpool.tile([128, 520], F16, tag=f"vn{tt}")
                vnat.append(vn)
                if "p" in parts:
                    thunks.append(
                        lambda tt=tt, vn=vn: emit_proj_group(
                            "v", tt, xt, vn))
            return thunks, (qkt, bdks, vnat)

        def emit_attn_scores(qkt, bdks, Ta, tt):
            # ---- attention: subtile tt covers windows 2tt, 2tt+1 of Ta.
            # HAZARD RULE: concurrent matmuls with disjoint row-groups but
            # a shared column-group collide in the PE array (device crash);
            # sub-128 matmuls are placed DIAGONALLY (out partition base ==
            # operand partition base). Scores land head-parity packed (e on
            # halves); the exp(Bbias)-multiply on DVE moves probs to
            # block-diagonal window-parity layout, so PV runs full-width
            # against natural V and outputs land in natural token rows.
            if True:
                if SSPLIT and SCORES_BD:
                    # separate PSUM tile per window parity so exp of parity
                    # p depends on only its own 4 matmuls even with
                    # tile-granular PSUM dependency tracking
                    s2 = []
                    for p in range(2):
                        s_half = (sps0 if p == 0 else sps).tile(
                            [128, 256], F32, tag=f"s{p}")
                        s2.append(s_half)
                    for p in range(2):
                        for j in range(4):
                            w = 2 * tt + p
                            nc.tensor.matmul(
                                s2[p][:, j * 64:(j + 1) * 64],
                                bdks[j][:, tt * 256 + p * 128:
                                        tt * 256 + (p + 1) * 128],
                                qkt["q", j][:, w * 64:(w + 1) * 64],
                                start=True, stop=True)
                    s = None
                else:
                    s = sps.tile([128, 512], F32, tag="s")
                if SCORES_BD and not SSPLIT:
                    # p-major: with ATTSPLIT, exp of parity p waits only on
                    # its own 4 matmuls
                    for p, j in (
                            [(p, j) for p in range(2) for j in range(4)]
                            if PMAJOR else
                            [(p, j) for j in range(4) for p in range(2)]):
                        if True:
                            w = 2 * tt + p
                            nc.tensor.matmul(
                                s[:, (j * 2 + p) * 64:(j * 2 + p + 1) * 64],
                                bdks[j][:, tt * 256 + p * 128:
                                        tt * 256 + (p + 1) * 128],
                                qkt["q", j][:, w * 64:(w + 1) * 64],
                                start=True, stop=True)
                elif not SCORES_BD:
                    for j in range(4):
                        for e in range(2):
                            r = slice(e * 64, e * 64 + 64)
                            for p in range(2):
                                w = 2 * tt + p
                                wc = slice(w * 64, w * 64 + 64)
                                nc.tensor.matmul(
                                    s[r, (j * 2 + p) * 64:
                                      (j * 2 + p + 1) * 64],
                                    qkt["k", j][r, wc],
                                    qkt["q", j][r, wc],
                                    start=True, stop=True)
                et = epool.tile([128, 512], F16, tag="et")
                et_v = et[:].rearrange("r (j u q) -> r j u q", u=2, q=64)
                if SSPLIT and SCORES_BD:
                    for p in range(2):
                        nc.scalar.activation(
                            et_v[:, :, p, :],
                            s2[p][:].rearrange("r (j q) -> r j q", q=64),
                            Exp)
                elif ATTSPLIT:
                    # split exp per window parity: the (p, e) multiplies
                    # depend only on their own exp half
                    s_v = s[:].rearrange("r (j u q) -> r j u q", u=2, q=64)
                    for p in range(2):
                        nc.scalar.activation(
                            et_v[:, :, p, :], s_v[:, :, p, :], Exp)
                else:
                    nc.scalar.activation(et[:], s[:], Exp)
                # block-diagonal probs: pt[p*64+k, h*128+p*64+q] =
                # et[e*64+k, (2j+p)*64+q]*ebt[k,q] (h=2j+e); off-diagonal
                # blocks stay zero, so one PV matmul covers both windows
                # with full 128-partition contraction against natural V.
                pt = epool.tile([128, 1024], F16, tag="pt")
                pt_v = pt[:].rearrange("r (j z) -> r j z", j=4)
                for p in range(2):
                    rp = slice(p * 64, p * 64 + 64)
                    for e in range(2):
                        re = slice(e * 64, e * 64 + 64)
                        c0 = e * 128 + p * 64
                        if ATTSPLIT:
                            # split per bank-half: PV bank b waits only on
                            # the j in {2b, 2b+1} multiplies
                            for bh in range(2):
                                js = slice(bh * 2, bh * 2 + 2)
                                nc.vector.tensor_mul(
                                    pt_v[rp, js, c0:c0 + 64],
                                    et_v[re, js, p, :],
                                    ebt_sb[re, 0:64].unsqueeze(1)
                                    .broadcast_to((64, 2, 64)))
                        else:
                            nc.vector.tensor_mul(
                                pt_v[rp, :, c0:c0 + 64],
                                et_v[re, :, p, :],
                                ebt_sb[re, 0:64].unsqueeze(1)
                                .broadcast_to((64, 4, 64)))
                return pt

        def emit_attn_pv(pt, vnat, Ta, tt):
            if True:
                # PV: 8 matmuls (one per head), full 128 partitions; two
                # PSUM banks of 4 [128q2w, 65] units each.
                on = onpool.tile([128, 512], F32, tag=f"on{tt % 2}")
                for b in range(2):
                    o = ops.tile([128, 260], F32, tag=f"ob{b}")
                    o_v = o[:].rearrange("r (u x) -> r u x", x=65)
                    for u in range(4):
                        h = 4 * b + u
                        nc.tensor.matmul(
                            o[:, u * 65:(u + 1) * 65],
                            pt[:, h * 128:(h + 1) * 128],
                            vnat[tt][:, h * 65:(h + 1) * 65],
                            start=True, stop=True)
                    rc = rcpool.tile([128, 4], F32, tag=f"rc{b}")
                    nc.vector.reciprocal(rc[:, 0:4], o_v[:, :, 64])
                    nc.vector.tensor_mul(
                        on[:].rearrange("r (b2 u q) -> r b2 u q", b2=2, q=64)
                        [:, b, :, :],
                        o_v[:, :, 0:64],
                        rc[:, 0:4].unsqueeze(2).broadcast_to((128, 4, 64)))
                if "s" in parts:
                    eng = nc.sync if STORE_SP else nc.scalar
                    eng.dma_start(
                        out[Ta * 512 + tt * 128: Ta * 512 + (tt + 1) * 128,
                            :],
                        on[:])

        # software pipeline driver: attention trails projections by
        # PIPELINE T-tiles; with INTERLEAVE, attention subtiles are emitted
        # between projection groups as scheduler priority hints.
        state = {}
        for T in range(NT + PIPELINE):
            pthunks = []
            if T < NT:
                pthunks, entry = proj_thunks(T)
                state[T] = entry
            athunks = []
            if T >= PIPELINE and "a" in parts:
                Ta = T - PIPELINE
                q_, b_, v_ = state.pop(Ta)
                for tt in range(NCHUNK):
                    cell = {}

                    def a_sc(tt=tt, q=q_, bb=b_, Ta=Ta, cell=cell):
                        cell["pt"] = emit_attn_scores(q, bb, Ta, tt)

                    def a_pv(tt=tt, v=v_, Ta=Ta, cell=cell):
                        emit_attn_pv(cell["pt"], v, Ta, tt)

                    if INTERLEAVE == 2:
                        athunks += [a_sc, a_pv]
                    else:
                        athunks.append(lambda a=a_sc, b=a_pv: (a(), b()))
            if INTERLEAVE and pthunks and athunks:
                for i, th in enumerate(pthunks):
                    th()
                    if INTERLEAVE == 2:
                        if i % 3 != 0 and athunks:
                            athunks.pop(0)()
                    elif i % 3 == 2 and athunks:
                        athunks.pop(0)()
            else:
                for th in pthunks:
                    th()
            for th in athunks:
                th()


def _legalize_sync(nc, max_waits=1):
    """Hoist excess semaphore waits into standalone same-engine
    EventSemaphore instructions. Engine instruction streams execute in
    order, so a wait carried by an immediately-preceding EventSemaphore is
    equivalent to a wait on the instruction itself — and the walrus build
    in this environment rejects instructions with more than one wait."""
    import bass_rust
    n_new = 0
    fn = nc.m.functions[0]
    for blk in fn.blocks:
        out = []
        changed = False
        for ins in blk.instructions:
            si = ins.sync_info
            waits = list(si.on_wait) if si and si.on_wait else []
            if len(waits) > max_waits:
                keep = waits[-max_waits:]
                for w in waits[:-max_waits]:
                    es = mybir.InstEventSemaphore(
                        name=f"esw-{n_new}-{ins.name}", ins=[], outs=[])
                    es.engine = ins.engine
                    es.sync_info = bass_rust.SyncInfo(on_wait=[w], on_update=[])
                    out.append(es)
                    n_new += 1
                ins.sync_info = bass_rust.SyncInfo(
                    on_wait=keep,
                    on_update=list(si.on_update) if si.on_update else [])
                changed = True
            out.append(ins)
        if changed:
            blk.instructions = out
    return n_new


def _build_model(with_bias, iters=1, parts="pas"):
    nc = bass.Bass("TRN2", target_bir_lowering=False, debug=False,
                   enable_partition_id=False)
    xT = nc.dram_tensor("xT", [512, 4096], F16, kind="ExternalInput").ap()
    wq = nc.dram_tensor("wq", [512, 512], F16, kind="ExternalInput").ap()
    wk = nc.dram_tensor("wk", [512, 512], F16, kind="ExternalInput").ap()
    wv = nc.dram_tensor("wv", [512, 512], F16, kind="ExternalInput").ap()
    ebt = nc.dram_tensor("ebt", [128, 64], F16, kind="ExternalInput").ap()
    bqk = (nc.dram_tensor("bqk", [128, 8], F32, kind="ExternalInput").ap()
           if with_bias else None)
    out = nc.dram_tensor("out", [4096, 512], F32, kind="ExternalOutput").ap()
    with tile.TileContext(nc) as tc:
        _emit(tc, out, xT, wq, wk, wv, ebt, bqk, iters=iters, parts=parts)
    return nc


_MODEL_CACHE = {}


def get_model(with_bias=False, legalize=True, iters=1, parts="pas"):
    key = (with_bias, legalize, iters, parts, SCORES_BD, PIPELINE, STORE_SP,
           PROJBUFS, OPSBUFS, EBUFS, INTERLEAVE, SPSBUFS, ATTSPLIT, PMAJOR,
           ONBUFS, SSPLIT, S0DOUBLE)
    if key not in _MODEL_CACHE:
        nc = _build_model(with_bias, iters=iters, parts=parts)
        if legalize:
            _legalize_sync(nc)
        _MODEL_CACHE[key] = nc
    return _MODEL_CACHE[key]


def make_in_maps(x, Wq, bq, Wk, bk, Wv, bv, Bbias):
    """Host-side sharding + layout prep. Returns (in_maps, with_bias)."""
    x = np.asarray(x, np.float32)
    with_bias = bool(np.any(bq) or np.any(bk))
    if np.any(bv):
        raise NotImplementedError("nonzero bv not supported")
    wq16 = np.ascontiguousarray(np.asarray(Wq, np.float32).T / 8.0).astype(np.float16)
    wk16 = np.ascontiguousarray(np.asarray(Wk, np.float32).T).astype(np.float16)
    wv16 = np.ascontiguousarray(np.asarray(Wv, np.float32).T).astype(np.float16)
    eb = np.exp(np.asarray(Bbias, np.float32).T)
    ebt = np.concatenate([eb, eb], 0).astype(np.float16)  # [128 (k x2), 64 q]
    common = {"wq": wq16, "wk": wk16, "wv": wv16, "ebt": ebt}
    if with_bias:
        bqk = np.concatenate(
            [np.asarray(bq, np.float32).reshape(4, 128).T / 8.0,
             np.asarray(bk, np.float32).reshape(4, 128).T], 1)  # [128, 8]
        common["bqk"] = np.ascontiguousarray(bqk)
    in_maps = []
    for b in range(B):
        xT16 = np.ascontiguousarray(
            x[b].reshape(TOK, C).T).astype(np.float16)
        in_maps.append({"xT": xT16, **common})
    return in_maps, with_bias


def kernel(**inputs):
    from concourse.bass_utils import run_bass_kernel_spmd
    in_maps, with_bias = make_in_maps(**inputs)
    nc = get_model(with_bias)
    res = run_bass_kernel_spmd(
        nc, in_maps, core_ids=list(range(B)), trace=TRACE)
    LAST["results"] = res
    out = np.stack([r["out"] for r in res.results], 0)
    return out.reshape(B, C, HH, WW)


def _harvest_io(nc):
    import jax
    in_names, out_names, out_avals = [], [], []
    for alloc in nc.m.functions[0].allocations:
        if not isinstance(alloc, mybir.MemoryLocationSet):
            continue
        name = alloc.memorylocations[0].name
        if alloc.kind == "ExternalInput":
            in_names.append(name)
        elif alloc.kind == "ExternalOutput":
            out_names.append(name)
            out_avals.append(jax.core.ShapedArray(
                tuple(alloc.tensor_shape), mybir.dt.np(alloc.dtype)))
    return in_names, out_names, out_avals


def _make_timed_callable(nc, in_maps):
    """Build a jitted shard_map callable around the single bass_exec of
    `nc` (mirrors run_bass_via_pjrt, but with NO donation so the same
    device-resident args can be reused across timed calls; outputs are
    garbage — timing only). Returns a zero-arg closure that runs one
    dispatch and blocks."""
    import jax
    from jax.sharding import Mesh, PartitionSpec
    from jax.experimental.shard_map import shard_map
    from concourse import bass2jax

    bass2jax.install_neuronx_cc_hook()
    in_names, out_names, out_avals = _harvest_io(nc)
    n_params = len(in_names)
    all_names = tuple(in_names + out_names)
    n_cores = len(in_maps)

    def _body(*args):
        return tuple(bass2jax._bass_exec_p.bind(
            *args,
            out_avals=tuple(out_avals),
            in_names=all_names,
            out_names=tuple(out_names),
            lowering_input_output_aliases=(),
            sim_require_finite=True,
            sim_require_nnan=True,
            nc=nc))

    devices = jax.devices()[:n_cores]
    mesh = Mesh(np.asarray(devices), ("core",))
    n_all = n_params + len(out_names)
    sharded = jax.jit(shard_map(
        _body, mesh=mesh,
        in_specs=(PartitionSpec("core"),) * n_all,
        out_specs=(PartitionSpec("core"),) * len(out_names),
        check_rep=False), keep_unused=True)
    concat_in = [
        np.concatenate([np.asarray(m[name]) for m in in_maps], 0)
        for name in in_names]
    concat_zeros = [
        np.zeros((n_cores * a.shape[0], *a.shape[1:]), a.dtype)
        for a in out_avals]
    args = [jax.device_put(a) for a in concat_in + concat_zeros]
    jax.block_until_ready(sharded(*args))  # warm-up / compile

    def run():
        jax.block_until_ready(sharded(*args))
    return run


def time_kernel(inputs, iters=4096, samples=8, parts="pas"):
    """Returns ns per iteration. Builds two model variants — the body run
    once vs `1+iters` times inside an on-device For_i loop — and
    differences median wall-clock over `samples` dispatches of each. With
    ~1s on-device per N-iter dispatch, the ~±20ms axon dispatch jitter
    contributes <2% error."""
    import time
    in_maps, with_bias = make_in_maps(**inputs)
    run1 = _make_timed_callable(
        get_model(with_bias, iters=1, parts=parts), in_maps)
    runN = _make_timed_callable(
        get_model(with_bias, iters=1 + iters, parts=parts), in_maps)
    t1s, tNs = [], []
    for _ in range(samples):
        t0 = time.time(); run1(); t1s.append(time.time() - t0)
        t0 = time.time(); runN(); tNs.append(time.time() - t0)
    t1 = float(np.median(t1s)); tN = float(np.median(tNs))
    return (tN - t1) / iters * 1e9, (t1s, tNs)

